# revision 19
# baseline (speedup 1.0000x reference)
"""Trainium2 Bass kernel: Bahdanau-attention decoder (attention + 4x LSTM + fc).

Contract: kernel(**inputs) takes the FULL unsharded inputs (as produced by
setup_inputs) and returns the full (logits, h) tuple, matching the reference.

Sharding (8 NeuronCores):
  - Attention is data-parallel over batch (16 rows/core). enc_output is staged
    host-side in transposed bf16 layout so the keys matmul contracts on the
    partition axis with no on-device transpose.
  - context is AllGathered (batch-major), then the 4 LSTM layers and the fc
    projection run tensor-parallel: each core owns a 128-wide slice of each
    LSTM gate and a 4000-wide slice of the vocab. h is kept transposed
    ([dims, batch]) so the AllGather's partition-axis concatenation rebuilds
    h^T directly. The forget gate is skipped entirely (c_prev == 0).
  - Matmul operands are bf16 (fp32 PE matmul runs at half rate); all
    accumulation, softmax and gate math stays fp32.
"""

import numpy as np
import ml_dtypes

import concourse.bass as bass
import concourse.tile as tile
from concourse import bacc, mybir
from concourse.bass_utils import run_bass_kernel_spmd
from concourse.masks import make_identity

# Problem dims (hardcoded per the grading contract).
B, S, U, E, V = 128, 256, 1024, 256, 32000
NC_N = 8
BS = B // NC_N          # 16  batch rows per core
BSS = BS * S            # 4096 source positions per core
PC = U // 128           # 8   partition chunks of U
GS = U // NC_N          # 128 per-core slice width of each LSTM gate
FCC = V // NC_N         # 4000 true fc columns per core
FCP = 4096              # padded fc columns
HALF = BS // 2          # 8 batch rows per keys half

F32 = mybir.dt.float32
BF16 = mybir.dt.bfloat16
I32 = mybir.dt.int32
AF = mybir.ActivationFunctionType
ALU = mybir.AluOpType
AX = mybir.AxisListType

_BF = ml_dtypes.bfloat16
_DEBUG = False


def _to_bf16(x: np.ndarray) -> np.ndarray:
    """Fast fp32 -> bf16 with round-to-nearest-even (numpy bit trick)."""
    x = np.ascontiguousarray(x, dtype=np.float32)
    v = x.view(np.uint32)
    out = ((v + 0x7FFF + ((v >> 16) & 1)) >> 16).astype(np.uint16)
    return out.view(_BF)


def _chunked(a_bf16: np.ndarray) -> np.ndarray:
    """[C*128, N] -> [128, C, N] so sbuf[p, c, n] = a[c*128+p, n]."""
    cN, n = a_bf16.shape
    c = cN // 128
    return np.ascontiguousarray(
        a_bf16.reshape(c, 128, n).transpose(1, 0, 2)
    )


def _build():
    nc = bacc.Bacc("TRN2", target_bir_lowering=False, debug=False,
                   num_devices=NC_N)

    # ---- per-core external inputs ----
    encT_d = nc.dram_tensor("encT", [128, PC, BSS], BF16, kind="ExternalInput")
    hidT_d = nc.dram_tensor("hidT", [128, PC, BS], BF16, kind="ExternalInput")
    w1_d = nc.dram_tensor("w1", [128, PC, U], BF16, kind="ExternalInput")
    w1b_d = nc.dram_tensor("w1b", [1, U], BF16, kind="ExternalInput")
    w2_d = nc.dram_tensor("w2", [128, PC, U], BF16, kind="ExternalInput")
    w2bT_d = nc.dram_tensor("w2bT", [128, PC], F32, kind="ExternalInput")
    vw_d = nc.dram_tensor("vw", [128, PC], BF16, kind="ExternalInput")
    emb_d = nc.dram_tensor("emb", [V, E], BF16, kind="ExternalInput")
    xidx_d = nc.dram_tensor("xidx", [B, 1], I32, kind="ExternalInput")
    lw_d = [
        nc.dram_tensor(f"lw{i}", [128, kc, 3 * GS], BF16, kind="ExternalInput")
        for i, kc in enumerate((10, 16, 16, 16), start=1)
    ]
    lb_d = [
        nc.dram_tensor(f"lb{i}", [128, 3], F32, kind="ExternalInput")
        for i in range(1, 5)
    ]
    fcw_d = nc.dram_tensor("fcw", [128, PC, FCP], BF16, kind="ExternalInput")
    fcb_d = nc.dram_tensor("fcb", [1, FCP], BF16, kind="ExternalInput")

    # ---- per-core external outputs ----
    logits_d = nc.dram_tensor("logits", [B, FCP], F32, kind="ExternalOutput")
    hsh_d = nc.dram_tensor("hsh", [GS, B], F32, kind="ExternalOutput")
    if _DEBUG:
        dbg_q_d = nc.dram_tensor("dbg_q", [BS, U], F32, kind="ExternalOutput")
        dbg_sc_d = nc.dram_tensor("dbg_sc", [BS, S], F32,
                                  kind="ExternalOutput")
        dbg_attn_d = nc.dram_tensor("dbg_attn", [BS, S], F32,
                                    kind="ExternalOutput")
        dbg_ctx_d = nc.dram_tensor("dbg_ctx", [BS, U], F32,
                                   kind="ExternalOutput")
        dbg_h1_d = nc.dram_tensor("dbg_h1", [128, PC * 128], F32,
                                  kind="ExternalOutput")
        dbg_emb_d = nc.dram_tensor("dbg_emb", [128, (E // 128) * B], F32,
                                   kind="ExternalOutput")
        dbg_tanh_d = nc.dram_tensor("dbg_tanh", [128, PC, HALF * S], BF16,
                                    kind="ExternalOutput")
        dbg_qtb_d = nc.dram_tensor("dbg_qtb", [128, PC, BS], F32,
                                   kind="ExternalOutput")

    # ---- collective bounce buffers ----
    cc_ctx_in = nc.dram_tensor("cc_ctx_in", [BS, U], F32)
    cc_ctx_out = nc.dram_tensor("cc_ctx_out", [B, U], F32, addr_space="Shared")
    cc_h_in = [nc.dram_tensor(f"cc_h_in{i}", [GS, B], F32) for i in range(1, 5)]
    cc_h_out = [
        nc.dram_tensor(f"cc_h_out{i}", [U, B], F32, addr_space="Shared")
        for i in range(1, 5)
    ]
    rgroups = [list(range(NC_N))]

    with tile.TileContext(nc) as tc:
        with tc.tile_pool(name="persist", bufs=1) as persist, \
             tc.tile_pool(name="work", bufs=1) as work:
            # ---- constants / small loads ----
            id128 = persist.tile([128, 128], F32)
            make_identity(nc, id128[:])
            id128b = persist.tile([128, 128], BF16)
            make_identity(nc, id128b[:])
            ones16 = persist.tile([1, BS], BF16)
            nc.gpsimd.memset(ones16[:], 1.0)
            ones128 = persist.tile([1, 128], BF16)
            nc.gpsimd.memset(ones128[:], 1.0)

            hidT = persist.tile([128, PC, BS], BF16)
            nc.sync.dma_start(hidT[:], hidT_d.ap())
            w1b = persist.tile([1, U], BF16)
            nc.sync.dma_start(w1b[:], w1b_d.ap())
            w2bT = persist.tile([128, PC], F32)
            nc.sync.dma_start(w2bT[:], w2bT_d.ap())
            vw = persist.tile([128, PC], BF16)
            nc.sync.dma_start(vw[:], vw_d.ap())
            xidx = persist.tile([B, 1], I32)
            nc.sync.dma_start(xidx[:], xidx_d.ap())
            lbs = []
            for i in range(4):
                lb = persist.tile([128, 3], F32, tag=f"lb{i}")
                nc.sync.dma_start(lb[:], lb_d[i].ap())
                lbs.append(lb)
            fcb = persist.tile([1, FCP], BF16)
            nc.sync.dma_start(fcb[:], fcb_d.ap())

            # embedding gather (natural) + PE transpose ->
            # embT[p, c, i] = emb[x_i, c*128+p]
            emb_nat = persist.tile([B, E], BF16)
            nc.gpsimd.indirect_dma_start(
                emb_nat[:], None, emb_d.ap(),
                bass.IndirectOffsetOnAxis(ap=xidx[:], axis=0))
            embT = persist.tile([128, E // 128, B], BF16)
            with tc.tile_pool(name="psE", bufs=2, space="PSUM") as psE:
                for c in range(E // 128):
                    ep = psE.tile([128, B], BF16, tag="ep")
                    nc.tensor.matmul(ep[:], emb_nat[:, c * 128:(c + 1) * 128],
                                     id128b[:], is_transpose=True)
                    nc.vector.tensor_copy(embT[:, c, :], ep[:])
            if _DEBUG:
                dbge = work.tile([128, (E // 128) * B], F32, tag="dbge")
                nc.vector.tensor_copy(
                    dbge[:].rearrange("p (c b) -> p c b", b=B), embT[:])
                nc.sync.dma_start(dbg_emb_d.ap(), dbge[:])

            # long-lived transposed activations (bf16 matmul operands)
            qTb = persist.tile([128, PC, BS], F32)      # q^T + W2_b
            ctxT_f = persist.tile([128, PC, BS], F32)   # context^T shard (f32)
            ctxT = persist.tile([128, PC, 128], BF16)   # full context^T
            hT = persist.tile([128, PC, 128], BF16, tag="hT")  # full h^T

            # attention working tiles
            sc_row = work.tile([1, BSS], F32)
            sc_bp = work.tile([BS, S], F32)
            mx = work.tile([BS, 1], F32)
            mxn = work.tile([BS, 1], F32)
            p_sm = work.tile([BS, S], F32)
            ssum = work.tile([BS, 1], F32)
            rsum = work.tile([BS, 1], F32)
            attn_f = work.tile([BS, S], F32)
            attn_bf = work.tile([BS, S], BF16)
            af_row = work.tile([1, BSS], BF16)
            attn_bc = work.tile([128, BSS], BF16)
            ctx_nat = work.tile([BS, U], F32)

            with tc.tile_pool(name="bigA", bufs=1) as bigA:
                w1 = bigA.tile([128, PC, U], BF16)
                for c in range(PC):
                    nc.sync.dma_start(w1[:, c, :], w1_d.ap()[:, c, :])
                w2 = bigA.tile([128, PC, U], BF16)
                for c in range(PC):
                    nc.sync.dma_start(w2[:, c, :], w2_d.ap()[:, c, :])
                encT = bigA.tile([128, PC, BSS], BF16)
                for b in range(BS):
                    nc.sync.dma_start(
                        encT[:, :, b * S:(b + 1) * S],
                        encT_d.ap()[:, :, b * S:(b + 1) * S],
                    )
                tanh_sb = bigA.tile([128, PC, HALF * S], BF16)

                # ---- phase A: q = hidden @ W1 + W1_b  (natural [16, U]) ----
                with tc.tile_pool(name="psA", bufs=1,
                                  space="PSUM") as psA, \
                     tc.tile_pool(name="sbA", bufs=1) as sbA:
                    q_ps = psA.tile([BS, U], F32)
                    for nb in range(2):
                        sl = slice(nb * 512, (nb + 1) * 512)
                        for k in range(PC):
                            nc.tensor.matmul(
                                q_ps[:, sl], hidT[:, k, :], w1[:, k, sl],
                                start=(k == 0), stop=False)
                        nc.tensor.matmul(
                            q_ps[:, sl], ones16[:], w1b[0:1, sl],
                            start=False, stop=True)
                    q_sb = sbA.tile([BS, U], F32)
                    nc.scalar.copy(q_sb[:], q_ps[:])
                    if _DEBUG:
                        nc.sync.dma_start(dbg_q_d.ap(), q_sb[:])
                    # qT chunks (+ W2_b per-partition) for the tanh bias
                    for c in range(PC):
                        qt_ps = psA.tile([128, BS], F32, tag="qt")
                        nc.tensor.matmul(
                            qt_ps[:], q_sb[:, c * 128:(c + 1) * 128],
                            id128[0:BS, 0:BS], is_transpose=True)
                        nc.vector.tensor_add(
                            qTb[:, c, :], qt_ps[:],
                            w2bT[:, c:c + 1].to_broadcast((128, BS)))

                # ---- phase B+C: keys^T -> tanh -> score, per half ----
                # Each matmul accumulation region is exactly one PSUM bank
                # (512 f32): start=True clears has_written bank-wide, so
                # interleaved groups must not share a bank.
                with tc.tile_pool(name="psK", bufs=3, space="PSUM") as psK, \
                     tc.tile_pool(name="psS", bufs=2, space="PSUM") as psS:
                    for half in range(2):
                        for m in range(PC):
                            for ti in range(2):   # 2 psum tiles per (half, m)
                                kps = psK.tile([128, 2, 512], F32, tag="kp")
                                for k in range(PC):
                                    for pp in range(2):   # batch pair
                                        b0 = half * HALF + ti * 4 + pp * 2
                                        nc.tensor.matmul(
                                            kps[:, pp, :],
                                            w2[:, k, m * 128:(m + 1) * 128],
                                            encT[:, k, b0 * S:(b0 + 2) * S],
                                            start=(k == 0), stop=(k == PC - 1))
                                for pp in range(2):
                                    for o in range(2):
                                        b_loc = ti * 4 + pp * 2 + o
                                        b = half * HALF + b_loc
                                        nc.scalar.activation(
                                            tanh_sb[:, m,
                                                    b_loc * S:(b_loc + 1) * S],
                                            kps[:, pp, o * S:(o + 1) * S],
                                            AF.Tanh, bias=qTb[:, m, b:b + 1])
                        # score for this half: V^T @ tanh
                        for nb in range(HALF * S // 512):
                            sp = psS.tile([1, 512], F32, tag="sp")
                            sl = slice(nb * 512, (nb + 1) * 512)
                            for k in range(PC):
                                nc.tensor.matmul(
                                    sp[:], vw[:, k:k + 1], tanh_sb[:, k, sl],
                                    start=(k == 0), stop=(k == PC - 1))
                            nc.scalar.copy(
                                sc_row[0:1, half * HALF * S + nb * 512:
                                       half * HALF * S + (nb + 1) * 512],
                                sp[:])
                        if _DEBUG and half == 0:
                            nc.sync.dma_start(dbg_tanh_d.ap(), tanh_sb[:])

                if _DEBUG:
                    nc.sync.dma_start(dbg_qtb_d.ap(), qTb[:])
                # ---- phase D: softmax over S per batch row ----
                nc.sync.dma_start(sc_bp[:], sc_row[0:1, :])
                nc.vector.reduce_max(out=mx[:], in_=sc_bp[:], axis=AX.X)
                nc.vector.tensor_scalar_mul(mxn[:], mx[:], -1.0)
                nc.scalar.activation(p_sm[:], sc_bp[:], AF.Exp,
                                     bias=mxn[:], accum_out=ssum[:])
                nc.vector.reciprocal(rsum[:], ssum[:])
                nc.vector.tensor_scalar_mul(attn_f[:], p_sm[:], rsum[:])
                nc.vector.tensor_copy(attn_bf[:], attn_f[:])
                nc.sync.dma_start(af_row[0:1, :], attn_bf[:])
                if _DEBUG:
                    nc.sync.dma_start(dbg_sc_d.ap(), sc_bp[:])
                    nc.sync.dma_start(dbg_attn_d.ap(), attn_f[:])

                # ---- phase E: context^T = sum_s attn * enc^T ----
                with tc.tile_pool(name="psC", bufs=2, space="PSUM") as psC, \
                     tc.tile_pool(name="prodp", bufs=1) as prodp:
                    for nb in range(BSS // 512):
                        bc_ps = psC.tile([128, 512], F32, tag="bc")
                        sl = slice(nb * 512, (nb + 1) * 512)
                        nc.tensor.matmul(bc_ps[:], ones128[:],
                                         af_row[0:1, sl])
                        nc.vector.tensor_copy(attn_bc[:, sl], bc_ps[:])
                    for c in range(PC):
                        prod = prodp.tile([128, BSS], BF16, tag="prod")
                        nc.vector.tensor_mul(prod[:], encT[:, c, :],
                                             attn_bc[:])
                        nc.vector.reduce_sum(
                            out=ctxT_f[:, c, :],
                            in_=prod[:].rearrange("p (b s) -> p b s", s=S),
                            axis=AX.X)
                    # shard -> natural [16, U] for the batch-major AllGather
                    for c in range(PC):
                        cn_ps = psC.tile([BS, 128], F32, tag="cn")
                        nc.tensor.matmul(cn_ps[:], ctxT_f[:, c, :],
                                         id128[:], is_transpose=True)
                        nc.vector.tensor_copy(
                            ctx_nat[:, c * 128:(c + 1) * 128], cn_ps[:])
                if _DEBUG:
                    nc.sync.dma_start(dbg_ctx_d.ap(), ctx_nat[:])
                nc.sync.dma_start(cc_ctx_in.ap(), ctx_nat[:])
                nc.gpsimd.collective_compute(
                    "AllGather", ALU.bypass, replica_groups=rgroups,
                    ins=[cc_ctx_in.ap().opt()], outs=[cc_ctx_out.ap().opt()])
                ctxn = work.tile([128, U], F32)
                nc.sync.dma_start(ctxn[:], cc_ctx_out.ap())
                with tc.tile_pool(name="psD", bufs=2, space="PSUM") as psD:
                    for c in range(PC):
                        ct_ps = psD.tile([128, 128], F32, tag="ct")
                        nc.tensor.matmul(ct_ps[:],
                                         ctxn[:, c * 128:(c + 1) * 128],
                                         id128[:], is_transpose=True)
                        nc.scalar.copy(ctxT[:, c, :], ct_ps[:])

            # ---- LSTM + fc phase (bigA space reused) ----
            with tc.tile_pool(name="bigB", bufs=1) as bigB, \
                 tc.tile_pool(name="lwork", bufs=1) as lwork:
                lws = []
                for i, kc in enumerate((10, 16, 16, 16)):
                    lw = bigB.tile([128, kc, 3 * GS], BF16, tag=f"lw{i}")
                    for k in range(kc):
                        nc.sync.dma_start(lw[:, k, :], lw_d[i].ap()[:, k, :])
                    lws.append(lw)
                fcw = bigB.tile([128, PC, FCP], BF16)
                for c in range(PC):
                    nc.sync.dma_start(fcw[:, c, :], fcw_d.ap()[:, c, :])

                for li in range(4):
                    kc = 10 if li == 0 else 16
                    xtail = embT if li == 0 else hT
                    with tc.tile_pool(name=f"psL{li}", bufs=1,
                                      space="PSUM") as psL:
                        z_ps = psL.tile([128, 3 * GS], F32, tag="z")
                        for k in range(kc):
                            rhs = lws[li][:, k, :]
                            lhsT = (ctxT[:, k, :] if k < PC
                                    else xtail[:, k - PC, :])
                            nc.tensor.matmul(z_ps[:], lhsT, rhs,
                                             start=(k == 0),
                                             stop=(k == kc - 1))
                        i_s = lwork.tile([128, GS], F32, tag="i_s")
                        g_t = lwork.tile([128, GS], F32, tag="g_t")
                        o_s = lwork.tile([128, GS], F32, tag="o_s")
                        c_f = lwork.tile([128, GS], F32, tag="c_f")
                        c_t = lwork.tile([128, GS], F32, tag="c_t")
                        h_n = lwork.tile([128, GS], F32, tag="h_n")
                        nc.scalar.activation(i_s[:], z_ps[:, 0:GS],
                                             AF.Sigmoid,
                                             bias=lbs[li][:, 0:1])
                        nc.scalar.activation(g_t[:], z_ps[:, GS:2 * GS],
                                             AF.Tanh, bias=lbs[li][:, 1:2])
                        nc.scalar.activation(o_s[:], z_ps[:, 2 * GS:3 * GS],
                                             AF.Sigmoid,
                                             bias=lbs[li][:, 2:3])
                        nc.vector.tensor_mul(c_f[:], i_s[:], g_t[:])
                        nc.scalar.activation(c_t[:], c_f[:], AF.Tanh)
                        nc.vector.tensor_mul(h_n[:], o_s[:], c_t[:])
                        # h^T shard for the partition-axis AllGather
                        ht_ps = psL.tile([128, 128], F32, tag="ht")
                        nc.tensor.matmul(ht_ps[:], h_n[:], id128[:],
                                         is_transpose=True)
                        hts = lwork.tile([128, 128], F32, tag="hts")
                        nc.vector.tensor_copy(hts[:], ht_ps[:])
                    nc.sync.dma_start(cc_h_in[li].ap(), hts[:])
                    if li == 3:
                        nc.sync.dma_start(hsh_d.ap(), hts[:])
                    nc.gpsimd.collective_compute(
                        "AllGather", ALU.bypass, replica_groups=rgroups,
                        ins=[cc_h_in[li].ap().opt()],
                        outs=[cc_h_out[li].ap().opt()])
                    htf = lwork.tile([128, PC, 128], F32, tag="htf")
                    nc.sync.dma_start(
                        htf[:],
                        cc_h_out[li].ap().rearrange("(c p) b -> p c b", p=128))
                    nc.vector.tensor_copy(hT[:], htf[:])
                    if _DEBUG and li == 0:
                        nc.sync.dma_start(
                            dbg_h1_d.ap().rearrange("p (c b) -> p c b", b=128),
                            htf[:])

                # ---- fc: logits = h @ fc_w + fc_b (natural output) ----
                lg_sb = lwork.tile([128, FCP], F32, tag="lg")
                with tc.tile_pool(name="psF", bufs=3, space="PSUM") as psF:
                    for nb in range(FCP // 512):
                        lg_ps = psF.tile([128, 512], F32, tag="lgp")
                        sl = slice(nb * 512, (nb + 1) * 512)
                        for k in range(PC):
                            nc.tensor.matmul(lg_ps[:], hT[:, k, :],
                                             fcw[:, k, sl],
                                             start=(k == 0), stop=False)
                        nc.tensor.matmul(lg_ps[:], ones128[:],
                                         fcb[0:1, sl], start=False, stop=True)
                        nc.scalar.copy(lg_sb[:, sl], lg_ps[:])
                        nc.sync.dma_start(logits_d.ap()[:, sl],
                                          lg_sb[:, sl])

    nc.compile()
    return nc


_NC_CACHE = None


def _get_nc():
    global _NC_CACHE
    if _NC_CACHE is None:
        _NC_CACHE = _build()
    return _NC_CACHE


def _prep_inputs(x, hidden, enc_output, W1_w, W1_b, W2_w, W2_b, V_w, V_b, emb,
                 l1_W, l1_b, l2_W, l2_b, l3_W, l3_b, l4_W, l4_b, fc_w, fc_b):
    """Host-side sharding/layout/casting. Returns in_maps for the 8 cores."""
    x = np.asarray(x).reshape(B).astype(np.int64)
    hidden = np.asarray(hidden, dtype=np.float32)
    enc_output = np.asarray(enc_output, dtype=np.float32)

    # shared (replicated) tensors
    w1 = _chunked(_to_bf16(W1_w))                  # [128, 8, U]
    w1b = _to_bf16(np.asarray(W1_b)).reshape(1, U)
    w2 = _chunked(_to_bf16(W2_w))
    w2bT = np.ascontiguousarray(
        np.asarray(W2_b, dtype=np.float32).reshape(PC, 128).T)
    vw = np.ascontiguousarray(
        _to_bf16(np.asarray(V_w).reshape(U)).reshape(PC, 128).T)
    # V_b shifts every score equally -> softmax-invariant; dropped.
    emb_bf = _to_bf16(emb)                          # [V, E]
    xw = np.ascontiguousarray(x.reshape(B, 1).astype(np.int32))
    fcb_full = np.asarray(fc_b, dtype=np.float32)

    lw_all, lb_all = [], []
    for W, bb in ((l1_W, l1_b), (l2_W, l2_b), (l3_W, l3_b), (l4_W, l4_b)):
        Wb = _to_bf16(W)
        lw_all.append(Wb)
        lb_all.append(np.asarray(bb, dtype=np.float32))

    in_maps = []
    for k in range(NC_N):
        bsl = slice(k * BS, (k + 1) * BS)
        enc_c = _to_bf16(enc_output[bsl]).reshape(BSS, U)
        encT = _chunked(np.ascontiguousarray(enc_c.T))      # [128, 8, 4096]
        hidT = _chunked(np.ascontiguousarray(
            _to_bf16(hidden[bsl]).T))                       # [128, 8, 16]

        m = {
            "encT": encT, "hidT": hidT,
            "w1": w1, "w1b": w1b, "w2": w2, "w2bT": w2bT, "vw": vw,
            "emb": emb_bf, "xidx": xw,
            "fcb": np.zeros((1, FCP), _BF),
        }
        m["fcb"][0, :FCC] = _to_bf16(fcb_full[k * FCC:(k + 1) * FCC])
        fcw_pad = np.zeros((U, FCP), _BF)
        fcw_pad[:, :FCC] = _to_bf16(
            np.asarray(fc_w)[:, k * FCC:(k + 1) * FCC])
        m["fcw"] = _chunked(fcw_pad)

        for i, (Wb, bb) in enumerate(zip(lw_all, lb_all), start=1):
            cols = np.concatenate(
                [Wb[:, g * U + k * GS:g * U + (k + 1) * GS]
                 for g in (0, 2, 3)], axis=1)               # i, g, o
            m[f"lw{i}"] = _chunked(np.ascontiguousarray(cols))
            m[f"lb{i}"] = np.ascontiguousarray(np.stack(
                [bb[g * U + k * GS:g * U + (k + 1) * GS]
                 for g in (0, 2, 3)], axis=1))              # [128, 3]
        in_maps.append(m)
    return in_maps


def _run(in_maps, trace=False):
    nc = _get_nc()
    return run_bass_kernel_spmd(
        nc, in_maps, core_ids=list(range(NC_N)), trace=trace)


def _assemble(results):
    logits = np.empty((B, V), np.float32)
    h = np.empty((B, U), np.float32)
    for k in range(NC_N):
        logits[:, k * FCC:(k + 1) * FCC] = results[k]["logits"][:, :FCC]
        h[:, k * GS:(k + 1) * GS] = results[k]["hsh"].T
    return logits, h


def kernel(**inputs):
    in_maps = _prep_inputs(**inputs)
    res = _run(in_maps, trace=False)
    return _assemble(res.results)


def kernel_traced(**inputs):
    """Like kernel() but with NTFF profiling; returns (outputs, exec_time_ns)."""
    in_maps = _prep_inputs(**inputs)
    res = _run(in_maps, trace=True)
    return _assemble(res.results), res.exec_time_ns


# revision 20
# speedup vs baseline: 1.0619x; 1.0619x over previous
"""Trainium2 Bass kernel: Bahdanau-attention decoder (attention + 4x LSTM + fc).

Contract: kernel(**inputs) takes the FULL unsharded inputs (as produced by
setup_inputs) and returns the full (logits, h) tuple, matching the reference.

Sharding (8 NeuronCores):
  - Attention is data-parallel over batch (16 rows/core). enc_output is staged
    host-side in transposed bf16 layout so the keys matmul contracts on the
    partition axis with no on-device transpose.
  - context is AllGathered (batch-major), then the 4 LSTM layers and the fc
    projection run tensor-parallel: each core owns a 128-wide slice of each
    LSTM gate and a 4000-wide slice of the vocab. h is kept transposed
    ([dims, batch]) so the AllGather's partition-axis concatenation rebuilds
    h^T directly. The forget gate is skipped entirely (c_prev == 0).
  - Matmul operands are bf16 (fp32 PE matmul runs at half rate); accumulation,
    softmax and gate math stays fp32.

Pipelining: the keys matmul runs column-block-major so each 512-column block
finishes all 8 u-chunks in sequence; its score matvec, exp (softmax without
the shift — scores are O(1), and softmax is shift-invariant so dropping the
max subtraction is exact) and the VectorE context reduction are interleaved
behind the PE stream. PSUM accumulation regions are always exactly one bank
(start=True clears has_written bank-wide).
"""

import numpy as np
import ml_dtypes

import concourse.bass as bass
import concourse.tile as tile
from concourse import bacc, mybir
from concourse.bass_utils import run_bass_kernel_spmd
from concourse.masks import make_identity

# Problem dims (hardcoded per the grading contract).
B, S, U, E, V = 128, 256, 1024, 256, 32000
NC_N = 8
BS = B // NC_N          # 16  batch rows per core
BSS = BS * S            # 4096 source positions per core
PC = U // 128           # 8   partition chunks of U
GS = U // NC_N          # 128 per-core slice width of each LSTM gate
FCC = V // NC_N         # 4000 true fc columns per core
FCP = 4096              # padded fc columns
CB = BSS // 512         # 8 column blocks (2 batch rows each)
CTXQ = 2                # column blocks per context batch

F32 = mybir.dt.float32
BF16 = mybir.dt.bfloat16
I32 = mybir.dt.int32
AF = mybir.ActivationFunctionType
ALU = mybir.AluOpType
AX = mybir.AxisListType

_BF = ml_dtypes.bfloat16
_DEBUG = False


def _to_bf16(x: np.ndarray) -> np.ndarray:
    """Fast fp32 -> bf16 with round-to-nearest-even (numpy bit trick)."""
    x = np.ascontiguousarray(x, dtype=np.float32)
    v = x.view(np.uint32)
    out = ((v + 0x7FFF + ((v >> 16) & 1)) >> 16).astype(np.uint16)
    return out.view(_BF)


def _chunked(a_bf16: np.ndarray) -> np.ndarray:
    """[C*128, N] -> [128, C, N] so sbuf[p, c, n] = a[c*128+p, n]."""
    cN, n = a_bf16.shape
    c = cN // 128
    return np.ascontiguousarray(
        a_bf16.reshape(c, 128, n).transpose(1, 0, 2)
    )


def _build():
    nc = bacc.Bacc("TRN2", target_bir_lowering=False, debug=False,
                   num_devices=NC_N)

    # ---- per-core external inputs ----
    encT_d = nc.dram_tensor("encT", [128, PC, BSS], BF16, kind="ExternalInput")
    hidT_d = nc.dram_tensor("hidT", [128, PC, BS], BF16, kind="ExternalInput")
    w1_d = nc.dram_tensor("w1", [128, PC, U], BF16, kind="ExternalInput")
    w1b_d = nc.dram_tensor("w1b", [1, U], BF16, kind="ExternalInput")
    w2_d = nc.dram_tensor("w2", [128, PC, U], BF16, kind="ExternalInput")
    w2bT_d = nc.dram_tensor("w2bT", [128, PC], F32, kind="ExternalInput")
    vw_d = nc.dram_tensor("vw", [128, PC], BF16, kind="ExternalInput")
    emb_d = nc.dram_tensor("emb", [V, E], BF16, kind="ExternalInput")
    xidx_d = nc.dram_tensor("xidx", [B, 1], I32, kind="ExternalInput")
    lw_d = [
        nc.dram_tensor(f"lw{i}", [128, kc, 3 * GS], BF16, kind="ExternalInput")
        for i, kc in enumerate((10, 16, 16, 16), start=1)
    ]
    lb_d = [
        nc.dram_tensor(f"lb{i}", [128, 3], F32, kind="ExternalInput")
        for i in range(1, 5)
    ]
    fcw_d = nc.dram_tensor("fcw", [128, PC, FCP], BF16, kind="ExternalInput")
    fcb_d = nc.dram_tensor("fcb", [1, FCP], BF16, kind="ExternalInput")

    # ---- per-core external outputs ----
    logits_d = nc.dram_tensor("logits", [B, FCP], F32, kind="ExternalOutput")
    hsh_d = nc.dram_tensor("hsh", [GS, B], F32, kind="ExternalOutput")
    if _DEBUG:
        dbg_q_d = nc.dram_tensor("dbg_q", [BS, U], F32, kind="ExternalOutput")
        dbg_ctx_d = nc.dram_tensor("dbg_ctx", [BS, U], F32,
                                   kind="ExternalOutput")
        dbg_h1_d = nc.dram_tensor("dbg_h1", [128, PC * 128], F32,
                                  kind="ExternalOutput")
        dbg_emb_d = nc.dram_tensor("dbg_emb", [128, (E // 128) * B], F32,
                                   kind="ExternalOutput")

    # ---- collective bounce buffers (bf16: payload feeds matmuls only) ----
    cc_ctx_in = nc.dram_tensor("cc_ctx_in", [BS, U], BF16)
    cc_ctx_out = nc.dram_tensor("cc_ctx_out", [B, U], BF16,
                                addr_space="Shared")
    cc_h_in = [nc.dram_tensor(f"cc_h_in{i}", [GS, B], BF16)
               for i in range(1, 5)]
    cc_h_out = [
        nc.dram_tensor(f"cc_h_out{i}", [U, B], BF16, addr_space="Shared")
        for i in range(1, 5)
    ]
    rgroups = [list(range(NC_N))]

    with tile.TileContext(nc) as tc:
        with tc.tile_pool(name="persist", bufs=1) as persist, \
             tc.tile_pool(name="work", bufs=1) as work:
            # ---- constants / small loads (issued in consumption order) ----
            id128 = persist.tile([128, 128], F32)
            make_identity(nc, id128[:])
            id128b = persist.tile([128, 128], BF16)
            make_identity(nc, id128b[:])
            ones16 = persist.tile([1, BS], BF16)
            nc.gpsimd.memset(ones16[:], 1.0)
            ones128 = persist.tile([1, 128], BF16)
            nc.gpsimd.memset(ones128[:], 1.0)

            hidT = persist.tile([128, PC, BS], BF16)
            nc.sync.dma_start(hidT[:], hidT_d.ap())
            w2bT = persist.tile([128, PC], F32)
            nc.sync.dma_start(w2bT[:], w2bT_d.ap())
            vw = persist.tile([128, PC], BF16)
            nc.sync.dma_start(vw[:], vw_d.ap())
            w1b = persist.tile([1, U], BF16)
            nc.sync.dma_start(w1b[:], w1b_d.ap())
            xidx = persist.tile([B, 1], I32)
            nc.sync.dma_start(xidx[:], xidx_d.ap())

            # long-lived transposed activations (bf16 matmul operands)
            qTb = persist.tile([128, PC, BS], F32)      # q^T + W2_b
            ctxTu = persist.tile([128, PC, BS], F32)    # unnormalized ctx^T
            ctxT = persist.tile([128, PC, 128], BF16)   # full context^T
            hT = persist.tile([128, PC, 128], BF16, tag="hT")  # full h^T
            embT = persist.tile([128, E // 128, B], BF16)

            # attention working tiles
            p_row = work.tile([1, BSS], BF16)           # exp(score), unnorm
            ssum_row = work.tile([1, BS], F32)
            ssumP = work.tile([BS, 1], F32)
            rsum = work.tile([BS, 1], F32)
            p_bc = work.tile([128, BSS], BF16)
            ctx_nat = work.tile([BS, U], F32)
            ctx_nat_bf = work.tile([BS, U], BF16)
            lbs = []
            for i in range(4):
                lb = persist.tile([128, 3], F32, tag=f"lb{i}")
                nc.sync.dma_start(lb[:], lb_d[i].ap())
                lbs.append(lb)
            fcb = persist.tile([1, FCP], BF16)
            nc.sync.dma_start(fcb[:], fcb_d.ap())

            with tc.tile_pool(name="bigA", bufs=1) as bigA:
                w2 = bigA.tile([128, PC, U], BF16)
                for c in range(PC):
                    nc.sync.dma_start(w2[:, c, :], w2_d.ap()[:, c, :])
                encT = bigA.tile([128, PC, BSS], BF16)
                for cb in range(CB):
                    sl = slice(cb * 512, (cb + 1) * 512)
                    nc.sync.dma_start(encT[:, :, sl], encT_d.ap()[:, :, sl])
                w1 = bigA.tile([128, PC, U], BF16)
                for c in range(PC):
                    nc.sync.dma_start(w1[:, c, :], w1_d.ap()[:, c, :])

                # embedding gather (natural) + PE transpose ->
                # embT[p, c, i] = emb[x_i, c*128+p]
                emb_nat = bigA.tile([B, E], BF16)
                nc.gpsimd.indirect_dma_start(
                    emb_nat[:], None, emb_d.ap(),
                    bass.IndirectOffsetOnAxis(ap=xidx[:], axis=0))

                # ---- phase A: q = hidden @ W1 + W1_b (natural [16, U]) ----
                with tc.tile_pool(name="psA", bufs=1, space="PSUM") as psA, \
                     tc.tile_pool(name="sbA", bufs=1) as sbA:
                    q_ps = psA.tile([BS, U], F32)
                    for nb in range(2):
                        sl = slice(nb * 512, (nb + 1) * 512)
                        for k in range(PC):
                            nc.tensor.matmul(
                                q_ps[:, sl], hidT[:, k, :], w1[:, k, sl],
                                start=(k == 0), stop=False)
                        nc.tensor.matmul(
                            q_ps[:, sl], ones16[:], w1b[0:1, sl],
                            start=False, stop=True)
                    q_sb = sbA.tile([BS, U], F32)
                    nc.scalar.copy(q_sb[:], q_ps[:])
                    if _DEBUG:
                        nc.sync.dma_start(dbg_q_d.ap(), q_sb[:])
                    # qT chunks (+ W2_b per-partition) for the tanh bias
                    for c in range(PC):
                        qt_ps = psA.tile([128, BS], F32, tag="qt")
                        nc.tensor.matmul(
                            qt_ps[:], q_sb[:, c * 128:(c + 1) * 128],
                            id128[0:BS, 0:BS], is_transpose=True)
                        nc.vector.tensor_add(
                            qTb[:, c, :], qt_ps[:],
                            w2bT[:, c:c + 1].to_broadcast((128, BS)))
                    # emb transpose rides in this psum pool too
                    for c in range(E // 128):
                        ep = psA.tile([128, B], BF16, tag="ep")
                        nc.tensor.matmul(
                            ep[:], emb_nat[:, c * 128:(c + 1) * 128],
                            id128b[:], is_transpose=True)
                        nc.vector.tensor_copy(embT[:, c, :], ep[:])
                if _DEBUG:
                    dbge = work.tile([128, (E // 128) * B], F32, tag="dbge")
                    nc.vector.tensor_copy(
                        dbge[:].rearrange("p (c b) -> p c b", b=B), embT[:])
                    nc.sync.dma_start(dbg_emb_d.ap(), dbge[:])

                # ---- fused keys -> tanh -> score -> exp -> context ----
                # column-block-major: each 512-col block (2 batch rows)
                # completes keys for all 8 u-chunks, then its score matvec
                # and exp run while the PE streams the next block. Context
                # (VectorE) runs per CTXQ blocks, hidden under the PE.
                with tc.tile_pool(name="psK", bufs=4, space="PSUM") as psK, \
                     tc.tile_pool(name="psS", bufs=2, space="PSUM") as psS, \
                     tc.tile_pool(name="psBC", bufs=2, space="PSUM") as psBC, \
                     tc.tile_pool(name="tanhp", bufs=3) as tanhp, \
                     tc.tile_pool(name="prodp", bufs=2) as prodp:
                    for cb in range(CB):
                        csl = slice(cb * 512, (cb + 1) * 512)
                        th = tanhp.tile([128, PC, 512], BF16, tag="th")
                        for m in range(PC):
                            kp = psK.tile([128, 512], F32, tag="kp")
                            for k in range(PC):
                                nc.tensor.matmul(
                                    kp[:], w2[:, k, m * 128:(m + 1) * 128],
                                    encT[:, k, csl],
                                    start=(k == 0), stop=(k == PC - 1))
                            for o in range(2):
                                b = cb * 2 + o
                                nc.scalar.activation(
                                    th[:, m, o * S:(o + 1) * S],
                                    kp[:, o * S:(o + 1) * S],
                                    AF.Tanh, bias=qTb[:, m, b:b + 1])
                        # score for this block
                        sp = psS.tile([1, 512], F32, tag="sp")
                        for k in range(PC):
                            nc.tensor.matmul(
                                sp[:], vw[:, k:k + 1], th[:, k, :],
                                start=(k == 0), stop=(k == PC - 1))
                        # exp (softmax shift dropped: shift-invariant) + sums
                        for o in range(2):
                            b = cb * 2 + o
                            nc.scalar.activation(
                                p_row[0:1, b * S:(b + 1) * S],
                                sp[0:1, o * S:(o + 1) * S], AF.Exp,
                                accum_out=ssum_row[0:1, b:b + 1])
                        # broadcast exp row across partitions (PE rank-1)
                        bc_ps = psBC.tile([128, 512], F32, tag="bc")
                        nc.tensor.matmul(bc_ps[:], ones128[:], p_row[0:1, csl])
                        nc.vector.tensor_copy(p_bc[:, csl], bc_ps[:])
                        # context contribution for the finished quarter
                        if (cb + 1) % CTXQ == 0:
                            qsl = slice((cb + 1 - CTXQ) * 512, (cb + 1) * 512)
                            bsl = slice((cb + 1 - CTXQ) * 2, (cb + 1) * 2)
                            for c in range(PC):
                                prod = prodp.tile([128, CTXQ * 512], BF16,
                                                  tag="prod")
                                nc.vector.tensor_mul(prod[:], encT[:, c, qsl],
                                                     p_bc[:, qsl])
                                nc.vector.reduce_sum(
                                    out=ctxTu[:, c, bsl],
                                    in_=prod[:].rearrange(
                                        "p (b s) -> p b s", s=S),
                                    axis=AX.X)

                # ---- normalize context + to natural layout + AllGather ----
                nc.sync.dma_start(ssumP[:], ssum_row[0:1, :])
                nc.vector.reciprocal(rsum[:], ssumP[:])
                with tc.tile_pool(name="psC", bufs=2, space="PSUM") as psC:
                    for c in range(PC):
                        cn_ps = psC.tile([BS, 128], F32, tag="cn")
                        nc.tensor.matmul(cn_ps[:], ctxTu[:, c, :],
                                         id128[:], is_transpose=True)
                        nc.vector.tensor_scalar_mul(
                            ctx_nat[:, c * 128:(c + 1) * 128], cn_ps[:],
                            rsum[:])
                    nc.vector.tensor_copy(ctx_nat_bf[:], ctx_nat[:])
                    if _DEBUG:
                        nc.sync.dma_start(dbg_ctx_d.ap(), ctx_nat[:])
                    nc.sync.dma_start(cc_ctx_in.ap(), ctx_nat_bf[:])
                    nc.gpsimd.collective_compute(
                        "AllGather", ALU.bypass, replica_groups=rgroups,
                        ins=[cc_ctx_in.ap().opt()],
                        outs=[cc_ctx_out.ap().opt()])
                    ctxn = work.tile([128, U], BF16)
                    nc.sync.dma_start(ctxn[:], cc_ctx_out.ap())
                    for c in range(PC):
                        ct_ps = psC.tile([128, 128], BF16, tag="ct")
                        nc.tensor.matmul(ct_ps[:],
                                         ctxn[:, c * 128:(c + 1) * 128],
                                         id128b[:], is_transpose=True)
                        nc.vector.tensor_copy(ctxT[:, c, :], ct_ps[:])

            # ---- LSTM + fc phase (bigA space reused) ----
            with tc.tile_pool(name="bigB", bufs=1) as bigB, \
                 tc.tile_pool(name="lwork", bufs=1) as lwork:
                lws = []
                for i, kc in enumerate((10, 16, 16, 16)):
                    lw = bigB.tile([128, kc, 3 * GS], BF16, tag=f"lw{i}")
                    for k in range(kc):
                        nc.sync.dma_start(lw[:, k, :], lw_d[i].ap()[:, k, :])
                    lws.append(lw)
                fcw = bigB.tile([128, PC, FCP], BF16)
                for c in range(PC):
                    nc.sync.dma_start(fcw[:, c, :], fcw_d.ap()[:, c, :])

                for li in range(4):
                    kc = 10 if li == 0 else 16
                    xtail = embT if li == 0 else hT
                    with tc.tile_pool(name=f"psL{li}", bufs=1,
                                      space="PSUM") as psL:
                        z_ps = psL.tile([128, 3 * GS], F32, tag="z")
                        for k in range(kc):
                            lhsT = (ctxT[:, k, :] if k < PC
                                    else xtail[:, k - PC, :])
                            nc.tensor.matmul(z_ps[:], lhsT, lws[li][:, k, :],
                                             start=(k == 0),
                                             stop=(k == kc - 1))
                        i_s = lwork.tile([128, GS], F32, tag="i_s")
                        g_t = lwork.tile([128, GS], F32, tag="g_t")
                        o_s = lwork.tile([128, GS], F32, tag="o_s")
                        c_f = lwork.tile([128, GS], F32, tag="c_f")
                        c_t = lwork.tile([128, GS], F32, tag="c_t")
                        h_n = lwork.tile([128, GS], F32, tag="h_n")
                        nc.scalar.activation(i_s[:], z_ps[:, 0:GS],
                                             AF.Sigmoid,
                                             bias=lbs[li][:, 0:1])
                        nc.scalar.activation(g_t[:], z_ps[:, GS:2 * GS],
                                             AF.Tanh, bias=lbs[li][:, 1:2])
                        nc.scalar.activation(o_s[:], z_ps[:, 2 * GS:3 * GS],
                                             AF.Sigmoid,
                                             bias=lbs[li][:, 2:3])
                        nc.vector.tensor_mul(c_f[:], i_s[:], g_t[:])
                        nc.scalar.activation(c_t[:], c_f[:], AF.Tanh)
                        nc.vector.tensor_mul(h_n[:], o_s[:], c_t[:])
                        # h^T shard for the partition-axis AllGather
                        ht_ps = psL.tile([128, 128], F32, tag="ht")
                        nc.tensor.matmul(ht_ps[:], h_n[:], id128[:],
                                         is_transpose=True)
                        hts_bf = lwork.tile([128, 128], BF16, tag="hts_bf")
                        nc.vector.tensor_copy(hts_bf[:], ht_ps[:])
                        if li == 3:
                            hts_f = lwork.tile([128, 128], F32, tag="hts_f")
                            nc.vector.tensor_copy(hts_f[:], ht_ps[:])
                            nc.sync.dma_start(hsh_d.ap(), hts_f[:])
                    nc.sync.dma_start(cc_h_in[li].ap(), hts_bf[:])
                    nc.gpsimd.collective_compute(
                        "AllGather", ALU.bypass, replica_groups=rgroups,
                        ins=[cc_h_in[li].ap().opt()],
                        outs=[cc_h_out[li].ap().opt()])
                    nc.sync.dma_start(
                        hT[:],
                        cc_h_out[li].ap().rearrange("(c p) b -> p c b", p=128))
                    if _DEBUG and li == 0:
                        dbgh = lwork.tile([128, PC * 128], F32, tag="dbgh")
                        nc.vector.tensor_copy(
                            dbgh[:].rearrange("p (c b) -> p c b", b=128),
                            hT[:])
                        nc.sync.dma_start(dbg_h1_d.ap(), dbgh[:])

                # ---- fc: logits = h @ fc_w + fc_b (natural output) ----
                lg_sb = lwork.tile([128, FCP], F32, tag="lg")
                with tc.tile_pool(name="psF", bufs=3, space="PSUM") as psF:
                    for nb in range(FCP // 512):
                        lg_ps = psF.tile([128, 512], F32, tag="lgp")
                        sl = slice(nb * 512, (nb + 1) * 512)
                        for k in range(PC):
                            nc.tensor.matmul(lg_ps[:], hT[:, k, :],
                                             fcw[:, k, sl],
                                             start=(k == 0), stop=False)
                        nc.tensor.matmul(lg_ps[:], ones128[:],
                                         fcb[0:1, sl], start=False, stop=True)
                        nc.scalar.copy(lg_sb[:, sl], lg_ps[:])
                        nc.sync.dma_start(logits_d.ap()[:, sl],
                                          lg_sb[:, sl])

    nc.compile()
    return nc


_NC_CACHE = None


def _get_nc():
    global _NC_CACHE
    if _NC_CACHE is None:
        _NC_CACHE = _build()
    return _NC_CACHE


def _prep_inputs(x, hidden, enc_output, W1_w, W1_b, W2_w, W2_b, V_w, V_b, emb,
                 l1_W, l1_b, l2_W, l2_b, l3_W, l3_b, l4_W, l4_b, fc_w, fc_b):
    """Host-side sharding/layout/casting. Returns in_maps for the 8 cores."""
    x = np.asarray(x).reshape(B).astype(np.int64)
    hidden = np.asarray(hidden, dtype=np.float32)
    enc_output = np.asarray(enc_output, dtype=np.float32)

    # shared (replicated) tensors
    w1 = _chunked(_to_bf16(W1_w))                  # [128, 8, U]
    w1b = _to_bf16(np.asarray(W1_b)).reshape(1, U)
    w2 = _chunked(_to_bf16(W2_w))
    w2bT = np.ascontiguousarray(
        np.asarray(W2_b, dtype=np.float32).reshape(PC, 128).T)
    vw = np.ascontiguousarray(
        _to_bf16(np.asarray(V_w).reshape(U)).reshape(PC, 128).T)
    # V_b shifts every score equally -> softmax-invariant; dropped.
    emb_bf = _to_bf16(emb)                          # [V, E]
    xw = np.ascontiguousarray(x.reshape(B, 1).astype(np.int32))
    fcb_full = np.asarray(fc_b, dtype=np.float32)

    lw_all, lb_all = [], []
    for W, bb in ((l1_W, l1_b), (l2_W, l2_b), (l3_W, l3_b), (l4_W, l4_b)):
        lw_all.append(_to_bf16(W))
        lb_all.append(np.asarray(bb, dtype=np.float32))

    in_maps = []
    for k in range(NC_N):
        bsl = slice(k * BS, (k + 1) * BS)
        enc_c = _to_bf16(enc_output[bsl]).reshape(BSS, U)
        encT = _chunked(np.ascontiguousarray(enc_c.T))      # [128, 8, 4096]
        hidT = _chunked(np.ascontiguousarray(
            _to_bf16(hidden[bsl]).T))                       # [128, 8, 16]

        m = {
            "encT": encT, "hidT": hidT,
            "w1": w1, "w1b": w1b, "w2": w2, "w2bT": w2bT, "vw": vw,
            "emb": emb_bf, "xidx": xw,
            "fcb": np.zeros((1, FCP), _BF),
        }
        m["fcb"][0, :FCC] = _to_bf16(fcb_full[k * FCC:(k + 1) * FCC])
        fcw_pad = np.zeros((U, FCP), _BF)
        fcw_pad[:, :FCC] = _to_bf16(
            np.asarray(fc_w)[:, k * FCC:(k + 1) * FCC])
        m["fcw"] = _chunked(fcw_pad)

        for i, (Wb, bb) in enumerate(zip(lw_all, lb_all), start=1):
            cols = np.concatenate(
                [Wb[:, g * U + k * GS:g * U + (k + 1) * GS]
                 for g in (0, 2, 3)], axis=1)               # i, g, o
            m[f"lw{i}"] = _chunked(np.ascontiguousarray(cols))
            m[f"lb{i}"] = np.ascontiguousarray(np.stack(
                [bb[g * U + k * GS:g * U + (k + 1) * GS]
                 for g in (0, 2, 3)], axis=1))              # [128, 3]
        in_maps.append(m)
    return in_maps


def _run(in_maps, trace=False):
    nc = _get_nc()
    return run_bass_kernel_spmd(
        nc, in_maps, core_ids=list(range(NC_N)), trace=trace)


def _assemble(results):
    logits = np.empty((B, V), np.float32)
    h = np.empty((B, U), np.float32)
    for k in range(NC_N):
        logits[:, k * FCC:(k + 1) * FCC] = results[k]["logits"][:, :FCC]
        h[:, k * GS:(k + 1) * GS] = results[k]["hsh"].T
    return logits, h


def kernel(**inputs):
    in_maps = _prep_inputs(**inputs)
    res = _run(in_maps, trace=False)
    return _assemble(res.results)


def kernel_traced(**inputs):
    """Like kernel() but with NTFF profiling; returns (outputs, exec_time_ns)."""
    in_maps = _prep_inputs(**inputs)
    res = _run(in_maps, trace=True)
    return _assemble(res.results), res.exec_time_ns


# revision 48
# speedup vs baseline: 1.1301x; 1.0642x over previous
"""Trainium2 Bass kernel: Bahdanau-attention decoder (attention + 4x LSTM + fc).

Contract: kernel(**inputs) takes the FULL unsharded inputs (as produced by
setup_inputs) and returns the full (logits, h) tuple, matching the reference.

Sharding (8 NeuronCores):
  - Attention is data-parallel over batch (16 rows/core). enc_output is staged
    host-side in transposed bf16 layout so the keys matmul contracts on the
    partition axis with no on-device transpose.
  - context is AllGathered (batch-major), then the 4 LSTM layers and the fc
    projection run tensor-parallel: each core owns a 128-wide slice of each
    LSTM gate and a 4000-wide slice of the vocab. h is kept transposed
    ([dims, batch]) so the AllGather's partition-axis concatenation rebuilds
    h^T directly. The forget gate is skipped entirely (c_prev == 0).
  - Matmul operands are bf16 (fp32 PE matmul runs at half rate); accumulation,
    softmax and gate math stays fp32.

Pipelining: the keys matmul runs column-block-major so each 512-column block
finishes all 8 u-chunks in sequence; its score matvec, exp (softmax without
the shift — scores are O(1), and softmax is shift-invariant so dropping the
max subtraction is exact) and the VectorE context reduction are interleaved
behind the PE stream. PSUM accumulation regions are always exactly one bank
(start=True clears has_written bank-wide).
"""

import numpy as np
import ml_dtypes

import concourse.bass as bass
import concourse.tile as tile
from concourse import bacc, mybir
from concourse.bass_utils import run_bass_kernel_spmd
from concourse.masks import make_identity

# Problem dims (hardcoded per the grading contract).
B, S, U, E, V = 128, 256, 1024, 256, 32000
NC_N = 8
BS = B // NC_N          # 16  batch rows per core
BSS = BS * S            # 4096 source positions per core
PC = U // 128           # 8   partition chunks of U
GS = U // NC_N          # 128 per-core slice width of each LSTM gate
FCC = V // NC_N         # 4000 true fc columns per core
FCP = 4096              # padded fc columns
CB = BSS // 512         # 8 column blocks (2 batch rows each)
CTXQ = 2                # column blocks per context batch

F32 = mybir.dt.float32
BF16 = mybir.dt.bfloat16
I32 = mybir.dt.int32
AF = mybir.ActivationFunctionType
ALU = mybir.AluOpType
AX = mybir.AxisListType

_BF = ml_dtypes.bfloat16
_DEBUG = False


def _to_bf16(x: np.ndarray) -> np.ndarray:
    """Fast fp32 -> bf16 with round-to-nearest-even (numpy bit trick)."""
    x = np.ascontiguousarray(x, dtype=np.float32)
    v = x.view(np.uint32)
    out = ((v + 0x7FFF + ((v >> 16) & 1)) >> 16).astype(np.uint16)
    return out.view(_BF)


def _chunked(a_bf16: np.ndarray) -> np.ndarray:
    """[C*128, N] -> [128, C, N] so sbuf[p, c, n] = a[c*128+p, n]."""
    cN, n = a_bf16.shape
    c = cN // 128
    return np.ascontiguousarray(
        a_bf16.reshape(c, 128, n).transpose(1, 0, 2)
    )


def _build():
    nc = bacc.Bacc("TRN2", target_bir_lowering=False, debug=False,
                   num_devices=NC_N)

    # ---- per-core external inputs ----
    encT_d = nc.dram_tensor("encT", [128, PC, BSS], BF16, kind="ExternalInput")
    hidT_d = nc.dram_tensor("hidT", [128, PC, BS], BF16, kind="ExternalInput")
    w1_d = nc.dram_tensor("w1", [128, PC, U], BF16, kind="ExternalInput")
    w1b_d = nc.dram_tensor("w1b", [1, U], BF16, kind="ExternalInput")
    w2_d = nc.dram_tensor("w2", [128, PC, U], BF16, kind="ExternalInput")
    w2bT_d = nc.dram_tensor("w2bT", [128, PC], F32, kind="ExternalInput")
    vw_d = nc.dram_tensor("vw", [128, PC], BF16, kind="ExternalInput")
    emb_d = nc.dram_tensor("emb", [V, E], BF16, kind="ExternalInput")
    xidx_d = nc.dram_tensor("xidx", [B, 1], I32, kind="ExternalInput")
    lw_d = [
        nc.dram_tensor(f"lw{i}", [128, kc, 3 * GS], BF16, kind="ExternalInput")
        for i, kc in enumerate((10, 16, 16, 16), start=1)
    ]
    lb_d = [
        nc.dram_tensor(f"lb{i}", [128, 3], F32, kind="ExternalInput")
        for i in range(1, 5)
    ]
    fcw_d = nc.dram_tensor("fcw", [128, PC, FCP], BF16, kind="ExternalInput")
    fcb_d = nc.dram_tensor("fcb", [1, FCP], BF16, kind="ExternalInput")

    # ---- per-core external outputs ----
    logits_d = nc.dram_tensor("logits", [B, FCP], F32, kind="ExternalOutput")
    hsh_d = nc.dram_tensor("hsh", [GS, B], F32, kind="ExternalOutput")
    if _DEBUG:
        dbg_q_d = nc.dram_tensor("dbg_q", [BS, U], F32, kind="ExternalOutput")
        dbg_ctx_d = nc.dram_tensor("dbg_ctx", [BS, U], F32,
                                   kind="ExternalOutput")
        dbg_h1_d = nc.dram_tensor("dbg_h1", [128, PC * 128], F32,
                                  kind="ExternalOutput")
        dbg_emb_d = nc.dram_tensor("dbg_emb", [128, (E // 128) * B], F32,
                                   kind="ExternalOutput")

    # ---- collective bounce buffers (bf16: payload feeds matmuls only) ----
    cc_ctx_in = nc.dram_tensor("cc_ctx_in", [BS, U], BF16)
    cc_ctx_out = nc.dram_tensor("cc_ctx_out", [B, U], BF16,
                                addr_space="Shared")
    cc_h_in = [nc.dram_tensor(f"cc_h_in{i}", [GS, B], BF16)
               for i in range(1, 5)]
    cc_h_out = [
        nc.dram_tensor(f"cc_h_out{i}", [U, B], BF16, addr_space="Shared")
        for i in range(1, 5)
    ]
    rgroups = [list(range(NC_N))]

    with tile.TileContext(nc) as tc:
        with tc.tile_pool(name="persist", bufs=1) as persist, \
             tc.tile_pool(name="work", bufs=1) as work:
            # ---- constants / small loads (issued in consumption order) ----
            id128 = persist.tile([128, 128], F32)
            make_identity(nc, id128[:])
            id128b = persist.tile([128, 128], BF16)
            make_identity(nc, id128b[:])
            ones16 = persist.tile([1, BS], BF16)
            nc.gpsimd.memset(ones16[:], 1.0)
            ones128 = persist.tile([1, 128], BF16)
            nc.gpsimd.memset(ones128[:], 1.0)

            hidT = persist.tile([128, PC, BS], BF16)
            nc.sync.dma_start(hidT[:], hidT_d.ap())
            w2bT = persist.tile([128, PC], F32)
            nc.sync.dma_start(w2bT[:], w2bT_d.ap())
            vw = persist.tile([128, PC], BF16)
            nc.sync.dma_start(vw[:], vw_d.ap())
            w1b = persist.tile([1, U], BF16)
            nc.sync.dma_start(w1b[:], w1b_d.ap())
            xidx = persist.tile([B, 1], I32)
            nc.sync.dma_start(xidx[:], xidx_d.ap())

            # long-lived transposed activations (bf16 matmul operands)
            qTb = persist.tile([128, PC, BS], F32)      # q^T + W2_b
            ctxTu = persist.tile([128, PC, BS], F32)    # unnormalized ctx^T
            ctxT = persist.tile([128, PC, 128], BF16)   # full context^T
            hT = persist.tile([128, PC, 128], BF16, tag="hT")  # full h^T
            embT = persist.tile([128, E // 128, B], BF16)

            # attention working tiles
            p_row = work.tile([1, BSS], BF16)           # exp(score), unnorm
            ssum_row = work.tile([1, BS], F32)
            ssumP = work.tile([BS, 1], F32)
            rsum = work.tile([BS, 1], F32)
            p_bc = work.tile([128, BSS], BF16)
            ctx_nat = work.tile([BS, U], F32)
            ctx_nat_bf = work.tile([BS, U], BF16)
            lbs = []
            for i in range(4):
                lb = persist.tile([128, 3], F32, tag=f"lb{i}")
                nc.sync.dma_start(lb[:], lb_d[i].ap())
                lbs.append(lb)
            fcb = persist.tile([1, FCP], BF16)
            nc.sync.dma_start(fcb[:], fcb_d.ap())

            with tc.tile_pool(name="bigA", bufs=1) as bigA:
                # w2/w1 first: q (hence the tanh bias) gates the keys
                # eviction pipeline, so it must not sit behind the 8 MB encT.
                w2 = bigA.tile([128, PC, U], BF16)
                for c in range(PC):
                    nc.sync.dma_start(w2[:, c, :], w2_d.ap()[:, c, :])
                w1 = bigA.tile([128, PC, U], BF16)
                for c in range(PC):
                    nc.sync.dma_start(w1[:, c, :], w1_d.ap()[:, c, :])
                encT = bigA.tile([128, PC, BSS], BF16)
                for cb in range(CB):
                    sl = slice(cb * 512, (cb + 1) * 512)
                    nc.sync.dma_start(encT[:, :, sl], encT_d.ap()[:, :, sl])

                # embedding gather (natural) + PE transpose ->
                # embT[p, c, i] = emb[x_i, c*128+p]
                emb_nat = bigA.tile([B, E], BF16)
                nc.gpsimd.indirect_dma_start(
                    emb_nat[:], None, emb_d.ap(),
                    bass.IndirectOffsetOnAxis(ap=xidx[:], axis=0))

                # ---- phase A: q = hidden @ W1 + W1_b (natural [16, U]) ----
                with tc.tile_pool(name="psA", bufs=1, space="PSUM") as psA, \
                     tc.tile_pool(name="sbA", bufs=1) as sbA:
                    q_ps = psA.tile([BS, U], F32)
                    for nb in range(2):
                        sl = slice(nb * 512, (nb + 1) * 512)
                        for k in range(PC):
                            nc.tensor.matmul(
                                q_ps[:, sl], hidT[:, k, :], w1[:, k, sl],
                                start=(k == 0), stop=False)
                        nc.tensor.matmul(
                            q_ps[:, sl], ones16[:], w1b[0:1, sl],
                            start=False, stop=True)
                    q_sb = sbA.tile([BS, U], F32)
                    nc.scalar.copy(q_sb[:], q_ps[:])
                    if _DEBUG:
                        nc.sync.dma_start(dbg_q_d.ap(), q_sb[:])
                    # qT chunks (+ W2_b per-partition) for the tanh bias
                    for c in range(PC):
                        qt_ps = psA.tile([128, BS], F32, tag="qt")
                        nc.tensor.matmul(
                            qt_ps[:], q_sb[:, c * 128:(c + 1) * 128],
                            id128[0:BS, 0:BS], is_transpose=True)
                        nc.vector.tensor_add(
                            qTb[:, c, :], qt_ps[:],
                            w2bT[:, c:c + 1].to_broadcast((128, BS)))
                    # emb transpose rides in this psum pool too
                    for c in range(E // 128):
                        ep = psA.tile([128, B], BF16, tag="ep")
                        nc.tensor.matmul(
                            ep[:], emb_nat[:, c * 128:(c + 1) * 128],
                            id128b[:], is_transpose=True)
                        nc.vector.tensor_copy(embT[:, c, :], ep[:])
                if _DEBUG:
                    dbge = work.tile([128, (E // 128) * B], F32, tag="dbge")
                    nc.vector.tensor_copy(
                        dbge[:].rearrange("p (c b) -> p c b", b=B), embT[:])
                    nc.sync.dma_start(dbg_emb_d.ap(), dbge[:])

                # ---- fused keys -> tanh -> score -> exp -> context ----
                # column-block-major: each 512-col block (2 batch rows)
                # completes keys for all 8 u-chunks, then its score matvec
                # and exp run while the PE streams the next block. Context
                # (VectorE) runs per CTXQ blocks, hidden under the PE.
                with tc.tile_pool(name="psK", bufs=4, space="PSUM") as psK, \
                     tc.tile_pool(name="psS", bufs=2, space="PSUM") as psS, \
                     tc.tile_pool(name="psBC", bufs=2, space="PSUM") as psBC, \
                     tc.tile_pool(name="tanhp", bufs=3) as tanhp, \
                     tc.tile_pool(name="prodp", bufs=2) as prodp:
                    for cb in range(CB):
                        csl = slice(cb * 512, (cb + 1) * 512)
                        th = tanhp.tile([128, PC, 512], BF16, tag="th")
                        for m in range(PC):
                            kp = psK.tile([128, 512], F32, tag="kp")
                            for k in range(PC):
                                nc.tensor.matmul(
                                    kp[:], w2[:, k, m * 128:(m + 1) * 128],
                                    encT[:, k, csl],
                                    start=(k == 0), stop=(k == PC - 1))
                            for o in range(2):
                                b = cb * 2 + o
                                nc.scalar.activation(
                                    th[:, m, o * S:(o + 1) * S],
                                    kp[:, o * S:(o + 1) * S],
                                    AF.Tanh, bias=qTb[:, m, b:b + 1])
                        # score for this block
                        sp = psS.tile([1, 512], F32, tag="sp")
                        for k in range(PC):
                            nc.tensor.matmul(
                                sp[:], vw[:, k:k + 1], th[:, k, :],
                                start=(k == 0), stop=(k == PC - 1))
                        # exp (softmax shift dropped: shift-invariant) + sums
                        for o in range(2):
                            b = cb * 2 + o
                            nc.scalar.activation(
                                p_row[0:1, b * S:(b + 1) * S],
                                sp[0:1, o * S:(o + 1) * S], AF.Exp)
                        nc.vector.reduce_sum(
                            out=ssum_row[0:1, cb * 2:cb * 2 + 2],
                            in_=p_row[0:1, csl].rearrange(
                                "p (b s) -> p b s", s=S),
                            axis=AX.X)
                        # broadcast exp row across partitions (PE rank-1)
                        bc_ps = psBC.tile([128, 512], F32, tag="bc")
                        nc.tensor.matmul(bc_ps[:], ones128[:], p_row[0:1, csl])
                        nc.vector.tensor_copy(p_bc[:, csl], bc_ps[:])
                        # context contribution for the finished quarter
                        if (cb + 1) % CTXQ == 0:
                            qsl = slice((cb + 1 - CTXQ) * 512, (cb + 1) * 512)
                            bsl = slice((cb + 1 - CTXQ) * 2, (cb + 1) * 2)
                            for c in range(PC):
                                prod = prodp.tile([128, CTXQ * 512], BF16,
                                                  tag="prod")
                                nc.vector.tensor_mul(prod[:], encT[:, c, qsl],
                                                     p_bc[:, qsl])
                                nc.vector.reduce_sum(
                                    out=ctxTu[:, c, bsl],
                                    in_=prod[:].rearrange(
                                        "p (b s) -> p b s", s=S),
                                    axis=AX.X)

                # ---- normalize context + to natural layout + AllGather ----
                nc.sync.dma_start(ssumP[:], ssum_row[0:1, :])
                nc.vector.reciprocal(rsum[:], ssumP[:])
                with tc.tile_pool(name="psC", bufs=2, space="PSUM") as psC:
                    for c in range(PC):
                        cn_ps = psC.tile([BS, 128], F32, tag="cn")
                        nc.tensor.matmul(cn_ps[:], ctxTu[:, c, :],
                                         id128[:], is_transpose=True)
                        nc.vector.tensor_scalar_mul(
                            ctx_nat[:, c * 128:(c + 1) * 128], cn_ps[:],
                            rsum[:])
                    nc.vector.tensor_copy(ctx_nat_bf[:], ctx_nat[:])
                    if _DEBUG:
                        nc.sync.dma_start(dbg_ctx_d.ap(), ctx_nat[:])
                    nc.sync.dma_start(cc_ctx_in.ap(), ctx_nat_bf[:])
                    nc.gpsimd.collective_compute(
                        "AllGather", ALU.bypass, replica_groups=rgroups,
                        ins=[cc_ctx_in.ap().opt()],
                        outs=[cc_ctx_out.ap().opt()])
                    ctxn = work.tile([128, U], BF16)
                    nc.sync.dma_start(ctxn[:], cc_ctx_out.ap())
                    for c in range(PC):
                        ct_ps = psC.tile([128, 128], BF16, tag="ct")
                        nc.tensor.matmul(ct_ps[:],
                                         ctxn[:, c * 128:(c + 1) * 128],
                                         id128b[:], is_transpose=True)
                        nc.vector.tensor_copy(ctxT[:, c, :], ct_ps[:])

            # ---- LSTM + fc phase (bigA space reused) ----
            with tc.tile_pool(name="bigB", bufs=1) as bigB, \
                 tc.tile_pool(name="lwork", bufs=1) as lwork:
                lws = []
                for i, kc in enumerate((10, 16, 16, 16)):
                    lw = bigB.tile([128, kc, 3 * GS], BF16, tag=f"lw{i}")
                    for k in range(kc):
                        nc.sync.dma_start(lw[:, k, :], lw_d[i].ap()[:, k, :])
                    lws.append(lw)
                fcw = bigB.tile([128, PC, FCP], BF16)
                for c in range(PC):
                    nc.sync.dma_start(fcw[:, c, :], fcw_d.ap()[:, c, :])

                for li in range(4):
                    kc = 10 if li == 0 else 16
                    xtail = embT if li == 0 else hT
                    with tc.tile_pool(name=f"psL{li}", bufs=1,
                                      space="PSUM") as psL:
                        z_ps = psL.tile([128, 3 * GS], F32, tag="z")
                        for k in range(kc):
                            lhsT = (ctxT[:, k, :] if k < PC
                                    else xtail[:, k - PC, :])
                            nc.tensor.matmul(z_ps[:], lhsT, lws[li][:, k, :],
                                             start=(k == 0),
                                             stop=(k == kc - 1))
                        i_s = lwork.tile([128, GS], F32, tag="i_s")
                        g_t = lwork.tile([128, GS], F32, tag="g_t")
                        o_s = lwork.tile([128, GS], F32, tag="o_s")
                        c_f = lwork.tile([128, GS], F32, tag="c_f")
                        c_t = lwork.tile([128, GS], F32, tag="c_t")
                        h_n = lwork.tile([128, GS], F32, tag="h_n")
                        nc.scalar.activation(i_s[:], z_ps[:, 0:GS],
                                             AF.Sigmoid,
                                             bias=lbs[li][:, 0:1])
                        nc.scalar.activation(g_t[:], z_ps[:, GS:2 * GS],
                                             AF.Tanh, bias=lbs[li][:, 1:2])
                        nc.scalar.activation(o_s[:], z_ps[:, 2 * GS:3 * GS],
                                             AF.Sigmoid,
                                             bias=lbs[li][:, 2:3])
                        nc.vector.tensor_mul(c_f[:], i_s[:], g_t[:])
                        nc.scalar.activation(c_t[:], c_f[:], AF.Tanh)
                        nc.vector.tensor_mul(h_n[:], o_s[:], c_t[:])
                        # h^T shard for the partition-axis AllGather
                        ht_ps = psL.tile([128, 128], F32, tag="ht")
                        nc.tensor.matmul(ht_ps[:], h_n[:], id128[:],
                                         is_transpose=True)
                        hts_bf = lwork.tile([128, 128], BF16, tag="hts_bf")
                        nc.vector.tensor_copy(hts_bf[:], ht_ps[:])
                        if li == 3:
                            hts_f = lwork.tile([128, 128], F32, tag="hts_f")
                            nc.vector.tensor_copy(hts_f[:], ht_ps[:])
                            nc.sync.dma_start(hsh_d.ap(), hts_f[:])
                    nc.sync.dma_start(cc_h_in[li].ap(), hts_bf[:])
                    nc.gpsimd.collective_compute(
                        "AllGather", ALU.bypass, replica_groups=rgroups,
                        ins=[cc_h_in[li].ap().opt()],
                        outs=[cc_h_out[li].ap().opt()])
                    nc.sync.dma_start(
                        hT[:],
                        cc_h_out[li].ap().rearrange("(c p) b -> p c b", p=128))
                    if _DEBUG and li == 0:
                        dbgh = lwork.tile([128, PC * 128], F32, tag="dbgh")
                        nc.vector.tensor_copy(
                            dbgh[:].rearrange("p (c b) -> p c b", b=128),
                            hT[:])
                        nc.sync.dma_start(dbg_h1_d.ap(), dbgh[:])

                # ---- fc: logits = h @ fc_w + fc_b (natural output) ----
                lg_sb = lwork.tile([128, FCP], F32, tag="lg")
                with tc.tile_pool(name="psF", bufs=3, space="PSUM") as psF:
                    for nb in range(FCP // 512):
                        lg_ps = psF.tile([128, 512], F32, tag="lgp")
                        sl = slice(nb * 512, (nb + 1) * 512)
                        for k in range(PC):
                            nc.tensor.matmul(lg_ps[:], hT[:, k, :],
                                             fcw[:, k, sl],
                                             start=(k == 0), stop=False)
                        nc.tensor.matmul(lg_ps[:], ones128[:],
                                         fcb[0:1, sl], start=False, stop=True)
                        nc.scalar.copy(lg_sb[:, sl], lg_ps[:])
                        nc.sync.dma_start(logits_d.ap()[:, sl],
                                          lg_sb[:, sl])

    nc.compile()
    return nc


_NC_CACHE = None


def _get_nc():
    global _NC_CACHE
    if _NC_CACHE is None:
        _NC_CACHE = _build()
    return _NC_CACHE


def _prep_inputs(x, hidden, enc_output, W1_w, W1_b, W2_w, W2_b, V_w, V_b, emb,
                 l1_W, l1_b, l2_W, l2_b, l3_W, l3_b, l4_W, l4_b, fc_w, fc_b):
    """Host-side sharding/layout/casting. Returns in_maps for the 8 cores."""
    x = np.asarray(x).reshape(B).astype(np.int64)
    hidden = np.asarray(hidden, dtype=np.float32)
    enc_output = np.asarray(enc_output, dtype=np.float32)

    # shared (replicated) tensors
    w1 = _chunked(_to_bf16(W1_w))                  # [128, 8, U]
    w1b = _to_bf16(np.asarray(W1_b)).reshape(1, U)
    w2 = _chunked(_to_bf16(W2_w))
    w2bT = np.ascontiguousarray(
        np.asarray(W2_b, dtype=np.float32).reshape(PC, 128).T)
    vw = np.ascontiguousarray(
        _to_bf16(np.asarray(V_w).reshape(U)).reshape(PC, 128).T)
    # V_b shifts every score equally -> softmax-invariant; dropped.
    emb_bf = _to_bf16(emb)                          # [V, E]
    xw = np.ascontiguousarray(x.reshape(B, 1).astype(np.int32))
    fcb_full = np.asarray(fc_b, dtype=np.float32)

    lw_all, lb_all = [], []
    for W, bb in ((l1_W, l1_b), (l2_W, l2_b), (l3_W, l3_b), (l4_W, l4_b)):
        lw_all.append(_to_bf16(W))
        lb_all.append(np.asarray(bb, dtype=np.float32))

    in_maps = []
    for k in range(NC_N):
        bsl = slice(k * BS, (k + 1) * BS)
        enc_c = _to_bf16(enc_output[bsl]).reshape(BSS, U)
        encT = _chunked(np.ascontiguousarray(enc_c.T))      # [128, 8, 4096]
        hidT = _chunked(np.ascontiguousarray(
            _to_bf16(hidden[bsl]).T))                       # [128, 8, 16]

        m = {
            "encT": encT, "hidT": hidT,
            "w1": w1, "w1b": w1b, "w2": w2, "w2bT": w2bT, "vw": vw,
            "emb": emb_bf, "xidx": xw,
            "fcb": np.zeros((1, FCP), _BF),
        }
        m["fcb"][0, :FCC] = _to_bf16(fcb_full[k * FCC:(k + 1) * FCC])
        fcw_pad = np.zeros((U, FCP), _BF)
        fcw_pad[:, :FCC] = _to_bf16(
            np.asarray(fc_w)[:, k * FCC:(k + 1) * FCC])
        m["fcw"] = _chunked(fcw_pad)

        for i, (Wb, bb) in enumerate(zip(lw_all, lb_all), start=1):
            cols = np.concatenate(
                [Wb[:, g * U + k * GS:g * U + (k + 1) * GS]
                 for g in (0, 2, 3)], axis=1)               # i, g, o
            m[f"lw{i}"] = _chunked(np.ascontiguousarray(cols))
            m[f"lb{i}"] = np.ascontiguousarray(np.stack(
                [bb[g * U + k * GS:g * U + (k + 1) * GS]
                 for g in (0, 2, 3)], axis=1))              # [128, 3]
        in_maps.append(m)
    return in_maps


def _run(in_maps, trace=False):
    nc = _get_nc()
    return run_bass_kernel_spmd(
        nc, in_maps, core_ids=list(range(NC_N)), trace=trace)


def _assemble(results):
    logits = np.empty((B, V), np.float32)
    h = np.empty((B, U), np.float32)
    for k in range(NC_N):
        logits[:, k * FCC:(k + 1) * FCC] = results[k]["logits"][:, :FCC]
        h[:, k * GS:(k + 1) * GS] = results[k]["hsh"].T
    return logits, h


def kernel(**inputs):
    in_maps = _prep_inputs(**inputs)
    res = _run(in_maps, trace=False)
    return _assemble(res.results)


def kernel_traced(**inputs):
    """Like kernel() but with NTFF profiling; returns (outputs, exec_time_ns)."""
    in_maps = _prep_inputs(**inputs)
    res = _run(in_maps, trace=True)
    return _assemble(res.results), res.exec_time_ns


# revision 50
# speedup vs baseline: 1.1346x; 1.0040x over previous
"""Trainium2 Bass kernel: Bahdanau-attention decoder (attention + 4x LSTM + fc).

Contract: kernel(**inputs) takes the FULL unsharded inputs (as produced by
setup_inputs) and returns the full (logits, h) tuple, matching the reference.

Sharding (8 NeuronCores):
  - Attention is data-parallel over batch (16 rows/core). enc_output is staged
    host-side in transposed bf16 layout so the keys matmul contracts on the
    partition axis with no on-device transpose.
  - context is AllGathered (batch-major), then the 4 LSTM layers and the fc
    projection run tensor-parallel: each core owns a 128-wide slice of each
    LSTM gate and a 4000-wide slice of the vocab. h is kept transposed
    ([dims, batch]) so the AllGather's partition-axis concatenation rebuilds
    h^T directly. The forget gate is skipped entirely (c_prev == 0).
  - Matmul operands are bf16 (fp32 PE matmul runs at half rate); accumulation,
    softmax and gate math stays fp32.

Pipelining: the keys matmul runs column-block-major so each 512-column block
finishes all 8 u-chunks in sequence; its score matvec, exp (softmax without
the shift — scores are O(1), and softmax is shift-invariant so dropping the
max subtraction is exact) and the VectorE context reduction are interleaved
behind the PE stream. PSUM accumulation regions are always exactly one bank
(start=True clears has_written bank-wide).
"""

import numpy as np
import ml_dtypes

import concourse.bass as bass
import concourse.tile as tile
from concourse import bacc, mybir
from concourse.bass_utils import run_bass_kernel_spmd
from concourse.masks import make_identity

# Problem dims (hardcoded per the grading contract).
B, S, U, E, V = 128, 256, 1024, 256, 32000
NC_N = 8
BS = B // NC_N          # 16  batch rows per core
BSS = BS * S            # 4096 source positions per core
PC = U // 128           # 8   partition chunks of U
GS = U // NC_N          # 128 per-core slice width of each LSTM gate
FCC = V // NC_N         # 4000 true fc columns per core
FCP = 4096              # padded fc columns
CB = BSS // 512         # 8 column blocks (2 batch rows each)
CTXQ = 2                # column blocks per context batch

F32 = mybir.dt.float32
BF16 = mybir.dt.bfloat16
I32 = mybir.dt.int32
AF = mybir.ActivationFunctionType
ALU = mybir.AluOpType
AX = mybir.AxisListType

_BF = ml_dtypes.bfloat16
_DEBUG = False


def _to_bf16(x: np.ndarray) -> np.ndarray:
    """Fast fp32 -> bf16 with round-to-nearest-even (numpy bit trick)."""
    x = np.ascontiguousarray(x, dtype=np.float32)
    v = x.view(np.uint32)
    out = ((v + 0x7FFF + ((v >> 16) & 1)) >> 16).astype(np.uint16)
    return out.view(_BF)


def _chunked(a_bf16: np.ndarray) -> np.ndarray:
    """[C*128, N] -> [128, C, N] so sbuf[p, c, n] = a[c*128+p, n]."""
    cN, n = a_bf16.shape
    c = cN // 128
    return np.ascontiguousarray(
        a_bf16.reshape(c, 128, n).transpose(1, 0, 2)
    )


def _build():
    nc = bacc.Bacc("TRN2", target_bir_lowering=False, debug=False,
                   num_devices=NC_N)

    # ---- per-core external inputs ----
    encT_d = nc.dram_tensor("encT", [128, PC, BSS], BF16, kind="ExternalInput")
    hidT_d = nc.dram_tensor("hidT", [128, PC, BS], BF16, kind="ExternalInput")
    w1_d = nc.dram_tensor("w1", [128, PC, U], BF16, kind="ExternalInput")
    w1b_d = nc.dram_tensor("w1b", [1, U], BF16, kind="ExternalInput")
    w2_d = nc.dram_tensor("w2", [128, PC, U], BF16, kind="ExternalInput")
    w2bT_d = nc.dram_tensor("w2bT", [128, PC], F32, kind="ExternalInput")
    vw_d = nc.dram_tensor("vw", [128, PC], BF16, kind="ExternalInput")
    emb_d = nc.dram_tensor("emb", [V, E], BF16, kind="ExternalInput")
    xidx_d = nc.dram_tensor("xidx", [B, 1], I32, kind="ExternalInput")
    lw_d = [
        nc.dram_tensor(f"lw{i}", [128, kc, 3 * GS], BF16, kind="ExternalInput")
        for i, kc in enumerate((10, 16, 16, 16), start=1)
    ]
    lb_d = [
        nc.dram_tensor(f"lb{i}", [128, 3], F32, kind="ExternalInput")
        for i in range(1, 5)
    ]
    fcw_d = nc.dram_tensor("fcw", [128, PC, FCP], BF16, kind="ExternalInput")
    fcb_d = nc.dram_tensor("fcb", [1, FCP], BF16, kind="ExternalInput")

    # ---- per-core external outputs ----
    logits_d = nc.dram_tensor("logits", [B, FCP], F32, kind="ExternalOutput")
    hsh_d = nc.dram_tensor("hsh", [GS, B], F32, kind="ExternalOutput")
    if _DEBUG:
        dbg_q_d = nc.dram_tensor("dbg_q", [BS, U], F32, kind="ExternalOutput")
        dbg_ctx_d = nc.dram_tensor("dbg_ctx", [BS, U], F32,
                                   kind="ExternalOutput")
        dbg_h1_d = nc.dram_tensor("dbg_h1", [128, PC * 128], F32,
                                  kind="ExternalOutput")
        dbg_emb_d = nc.dram_tensor("dbg_emb", [128, (E // 128) * B], F32,
                                   kind="ExternalOutput")

    # ---- collective bounce buffers (bf16: payload feeds matmuls only) ----
    cc_warm_in = nc.dram_tensor("cc_warm_in", [1, 128], F32)
    cc_warm_out = nc.dram_tensor("cc_warm_out", [NC_N, 128], F32,
                                 addr_space="Shared")
    cc_ctx_in = nc.dram_tensor("cc_ctx_in", [BS, U], BF16)
    cc_ctx_out = nc.dram_tensor("cc_ctx_out", [B, U], BF16,
                                addr_space="Shared")
    cc_h_in = [nc.dram_tensor(f"cc_h_in{i}", [GS, B], BF16)
               for i in range(1, 5)]
    cc_h_out = [
        nc.dram_tensor(f"cc_h_out{i}", [U, B], BF16, addr_space="Shared")
        for i in range(1, 5)
    ]
    rgroups = [list(range(NC_N))]

    with tile.TileContext(nc) as tc:
        with tc.tile_pool(name="persist", bufs=1) as persist, \
             tc.tile_pool(name="work", bufs=1) as work:
            # ---- constants / small loads (issued in consumption order) ----
            id128 = persist.tile([128, 128], F32)
            make_identity(nc, id128[:])
            id128b = persist.tile([128, 128], BF16)
            make_identity(nc, id128b[:])
            ones16 = persist.tile([1, BS], BF16)
            nc.gpsimd.memset(ones16[:], 1.0)
            ones128 = persist.tile([1, 128], BF16)
            nc.gpsimd.memset(ones128[:], 1.0)

            # warm up the collective path while the big DMAs stream in
            warm_sb = persist.tile([1, 128], F32)
            nc.gpsimd.memset(warm_sb[:], 0.0)
            nc.sync.dma_start(cc_warm_in.ap(), warm_sb[:])
            nc.gpsimd.collective_compute(
                "AllGather", ALU.bypass, replica_groups=rgroups,
                ins=[cc_warm_in.ap().opt()], outs=[cc_warm_out.ap().opt()])

            hidT = persist.tile([128, PC, BS], BF16)
            nc.sync.dma_start(hidT[:], hidT_d.ap())
            w2bT = persist.tile([128, PC], F32)
            nc.sync.dma_start(w2bT[:], w2bT_d.ap())
            vw = persist.tile([128, PC], BF16)
            nc.sync.dma_start(vw[:], vw_d.ap())
            w1b = persist.tile([1, U], BF16)
            nc.sync.dma_start(w1b[:], w1b_d.ap())
            xidx = persist.tile([B, 1], I32)
            nc.sync.dma_start(xidx[:], xidx_d.ap())

            # long-lived transposed activations (bf16 matmul operands)
            qTb = persist.tile([128, PC, BS], F32)      # q^T + W2_b
            ctxTu = persist.tile([128, PC, BS], F32)    # unnormalized ctx^T
            ctxT = persist.tile([128, PC, 128], BF16)   # full context^T
            hT = persist.tile([128, PC, 128], BF16, tag="hT")  # full h^T
            embT = persist.tile([128, E // 128, B], BF16)

            # attention working tiles
            p_row = work.tile([1, BSS], BF16)           # exp(score), unnorm
            ssum_row = work.tile([1, BS], F32)
            ssumP = work.tile([BS, 1], F32)
            rsum = work.tile([BS, 1], F32)
            p_bc = work.tile([128, BSS], BF16)
            ctx_nat = work.tile([BS, U], F32)
            ctx_nat_bf = work.tile([BS, U], BF16)
            lbs = []
            for i in range(4):
                lb = persist.tile([128, 3], F32, tag=f"lb{i}")
                nc.sync.dma_start(lb[:], lb_d[i].ap())
                lbs.append(lb)
            fcb = persist.tile([1, FCP], BF16)
            nc.sync.dma_start(fcb[:], fcb_d.ap())

            with tc.tile_pool(name="bigA", bufs=1) as bigA:
                # w2/w1 first: q (hence the tanh bias) gates the keys
                # eviction pipeline, so it must not sit behind the 8 MB encT.
                w2 = bigA.tile([128, PC, U], BF16)
                for c in range(PC):
                    nc.sync.dma_start(w2[:, c, :], w2_d.ap()[:, c, :])
                w1 = bigA.tile([128, PC, U], BF16)
                for c in range(PC):
                    nc.sync.dma_start(w1[:, c, :], w1_d.ap()[:, c, :])
                encT = bigA.tile([128, PC, BSS], BF16)
                for cb in range(CB):
                    sl = slice(cb * 512, (cb + 1) * 512)
                    nc.sync.dma_start(encT[:, :, sl], encT_d.ap()[:, :, sl])

                # embedding gather (natural) + PE transpose ->
                # embT[p, c, i] = emb[x_i, c*128+p]
                emb_nat = bigA.tile([B, E], BF16)
                nc.gpsimd.indirect_dma_start(
                    emb_nat[:], None, emb_d.ap(),
                    bass.IndirectOffsetOnAxis(ap=xidx[:], axis=0))

                # ---- phase A: q = hidden @ W1 + W1_b (natural [16, U]) ----
                with tc.tile_pool(name="psA", bufs=1, space="PSUM") as psA, \
                     tc.tile_pool(name="sbA", bufs=1) as sbA:
                    q_ps = psA.tile([BS, U], F32)
                    for nb in range(2):
                        sl = slice(nb * 512, (nb + 1) * 512)
                        for k in range(PC):
                            nc.tensor.matmul(
                                q_ps[:, sl], hidT[:, k, :], w1[:, k, sl],
                                start=(k == 0), stop=False)
                        nc.tensor.matmul(
                            q_ps[:, sl], ones16[:], w1b[0:1, sl],
                            start=False, stop=True)
                    q_sb = sbA.tile([BS, U], F32)
                    nc.scalar.copy(q_sb[:], q_ps[:])
                    if _DEBUG:
                        nc.sync.dma_start(dbg_q_d.ap(), q_sb[:])
                    # qT chunks (+ W2_b per-partition) for the tanh bias
                    for c in range(PC):
                        qt_ps = psA.tile([128, BS], F32, tag="qt")
                        nc.tensor.matmul(
                            qt_ps[:], q_sb[:, c * 128:(c + 1) * 128],
                            id128[0:BS, 0:BS], is_transpose=True)
                        nc.vector.tensor_add(
                            qTb[:, c, :], qt_ps[:],
                            w2bT[:, c:c + 1].to_broadcast((128, BS)))
                    # emb transpose rides in this psum pool too
                    for c in range(E // 128):
                        ep = psA.tile([128, B], BF16, tag="ep")
                        nc.tensor.matmul(
                            ep[:], emb_nat[:, c * 128:(c + 1) * 128],
                            id128b[:], is_transpose=True)
                        nc.vector.tensor_copy(embT[:, c, :], ep[:])
                if _DEBUG:
                    dbge = work.tile([128, (E // 128) * B], F32, tag="dbge")
                    nc.vector.tensor_copy(
                        dbge[:].rearrange("p (c b) -> p c b", b=B), embT[:])
                    nc.sync.dma_start(dbg_emb_d.ap(), dbge[:])

                # ---- fused keys -> tanh -> score -> exp -> context ----
                # column-block-major: each 512-col block (2 batch rows)
                # completes keys for all 8 u-chunks, then its score matvec
                # and exp run while the PE streams the next block. Context
                # (VectorE) runs per CTXQ blocks, hidden under the PE.
                with tc.tile_pool(name="psK", bufs=4, space="PSUM") as psK, \
                     tc.tile_pool(name="psS", bufs=2, space="PSUM") as psS, \
                     tc.tile_pool(name="psBC", bufs=2, space="PSUM") as psBC, \
                     tc.tile_pool(name="tanhp", bufs=3) as tanhp, \
                     tc.tile_pool(name="prodp", bufs=2) as prodp:
                    for cb in range(CB):
                        csl = slice(cb * 512, (cb + 1) * 512)
                        th = tanhp.tile([128, PC, 512], BF16, tag="th")
                        for m in range(PC):
                            kp = psK.tile([128, 512], F32, tag="kp")
                            for k in range(PC):
                                nc.tensor.matmul(
                                    kp[:], w2[:, k, m * 128:(m + 1) * 128],
                                    encT[:, k, csl],
                                    start=(k == 0), stop=(k == PC - 1))
                            for o in range(2):
                                b = cb * 2 + o
                                nc.scalar.activation(
                                    th[:, m, o * S:(o + 1) * S],
                                    kp[:, o * S:(o + 1) * S],
                                    AF.Tanh, bias=qTb[:, m, b:b + 1])
                        # score for this block
                        sp = psS.tile([1, 512], F32, tag="sp")
                        for k in range(PC):
                            nc.tensor.matmul(
                                sp[:], vw[:, k:k + 1], th[:, k, :],
                                start=(k == 0), stop=(k == PC - 1))
                        # exp (softmax shift dropped: shift-invariant) + sums
                        for o in range(2):
                            b = cb * 2 + o
                            nc.scalar.activation(
                                p_row[0:1, b * S:(b + 1) * S],
                                sp[0:1, o * S:(o + 1) * S], AF.Exp)
                        nc.vector.reduce_sum(
                            out=ssum_row[0:1, cb * 2:cb * 2 + 2],
                            in_=p_row[0:1, csl].rearrange(
                                "p (b s) -> p b s", s=S),
                            axis=AX.X)
                        # broadcast exp row across partitions (PE rank-1)
                        bc_ps = psBC.tile([128, 512], F32, tag="bc")
                        nc.tensor.matmul(bc_ps[:], ones128[:], p_row[0:1, csl])
                        nc.vector.tensor_copy(p_bc[:, csl], bc_ps[:])
                        # context contribution for the finished quarter
                        if (cb + 1) % CTXQ == 0:
                            qsl = slice((cb + 1 - CTXQ) * 512, (cb + 1) * 512)
                            bsl = slice((cb + 1 - CTXQ) * 2, (cb + 1) * 2)
                            for c in range(PC):
                                prod = prodp.tile([128, CTXQ * 512], BF16,
                                                  tag="prod")
                                nc.vector.tensor_mul(prod[:], encT[:, c, qsl],
                                                     p_bc[:, qsl])
                                nc.vector.reduce_sum(
                                    out=ctxTu[:, c, bsl],
                                    in_=prod[:].rearrange(
                                        "p (b s) -> p b s", s=S),
                                    axis=AX.X)

                # ---- normalize context + to natural layout + AllGather ----
                nc.sync.dma_start(ssumP[:], ssum_row[0:1, :])
                nc.vector.reciprocal(rsum[:], ssumP[:])
                with tc.tile_pool(name="psC", bufs=2, space="PSUM") as psC:
                    for c in range(PC):
                        cn_ps = psC.tile([BS, 128], F32, tag="cn")
                        nc.tensor.matmul(cn_ps[:], ctxTu[:, c, :],
                                         id128[:], is_transpose=True)
                        nc.vector.tensor_scalar_mul(
                            ctx_nat[:, c * 128:(c + 1) * 128], cn_ps[:],
                            rsum[:])
                    nc.vector.tensor_copy(ctx_nat_bf[:], ctx_nat[:])
                    if _DEBUG:
                        nc.sync.dma_start(dbg_ctx_d.ap(), ctx_nat[:])
                    nc.sync.dma_start(cc_ctx_in.ap(), ctx_nat_bf[:])
                    nc.gpsimd.collective_compute(
                        "AllGather", ALU.bypass, replica_groups=rgroups,
                        ins=[cc_ctx_in.ap().opt()],
                        outs=[cc_ctx_out.ap().opt()])
                    ctxn = work.tile([128, U], BF16)
                    nc.sync.dma_start(ctxn[:], cc_ctx_out.ap())
                    for c in range(PC):
                        ct_ps = psC.tile([128, 128], BF16, tag="ct")
                        nc.tensor.matmul(ct_ps[:],
                                         ctxn[:, c * 128:(c + 1) * 128],
                                         id128b[:], is_transpose=True)
                        nc.vector.tensor_copy(ctxT[:, c, :], ct_ps[:])

            # ---- LSTM + fc phase (bigA space reused) ----
            with tc.tile_pool(name="bigB", bufs=1) as bigB, \
                 tc.tile_pool(name="lwork", bufs=1) as lwork:
                lws = []
                for i, kc in enumerate((10, 16, 16, 16)):
                    lw = bigB.tile([128, kc, 3 * GS], BF16, tag=f"lw{i}")
                    for k in range(kc):
                        nc.sync.dma_start(lw[:, k, :], lw_d[i].ap()[:, k, :])
                    lws.append(lw)
                fcw = bigB.tile([128, PC, FCP], BF16)
                for c in range(PC):
                    nc.sync.dma_start(fcw[:, c, :], fcw_d.ap()[:, c, :])

                for li in range(4):
                    kc = 10 if li == 0 else 16
                    xtail = embT if li == 0 else hT
                    with tc.tile_pool(name=f"psL{li}", bufs=1,
                                      space="PSUM") as psL:
                        z_ps = psL.tile([128, 3 * GS], F32, tag="z")
                        for k in range(kc):
                            lhsT = (ctxT[:, k, :] if k < PC
                                    else xtail[:, k - PC, :])
                            nc.tensor.matmul(z_ps[:], lhsT, lws[li][:, k, :],
                                             start=(k == 0),
                                             stop=(k == kc - 1))
                        i_s = lwork.tile([128, GS], F32, tag="i_s")
                        g_t = lwork.tile([128, GS], F32, tag="g_t")
                        o_s = lwork.tile([128, GS], F32, tag="o_s")
                        c_f = lwork.tile([128, GS], F32, tag="c_f")
                        c_t = lwork.tile([128, GS], F32, tag="c_t")
                        h_n = lwork.tile([128, GS], F32, tag="h_n")
                        nc.scalar.activation(i_s[:], z_ps[:, 0:GS],
                                             AF.Sigmoid,
                                             bias=lbs[li][:, 0:1])
                        nc.scalar.activation(g_t[:], z_ps[:, GS:2 * GS],
                                             AF.Tanh, bias=lbs[li][:, 1:2])
                        nc.scalar.activation(o_s[:], z_ps[:, 2 * GS:3 * GS],
                                             AF.Sigmoid,
                                             bias=lbs[li][:, 2:3])
                        nc.vector.tensor_mul(c_f[:], i_s[:], g_t[:])
                        nc.scalar.activation(c_t[:], c_f[:], AF.Tanh)
                        nc.vector.tensor_mul(h_n[:], o_s[:], c_t[:])
                        # h^T shard for the partition-axis AllGather
                        ht_ps = psL.tile([128, 128], F32, tag="ht")
                        nc.tensor.matmul(ht_ps[:], h_n[:], id128[:],
                                         is_transpose=True)
                        hts_bf = lwork.tile([128, 128], BF16, tag="hts_bf")
                        nc.vector.tensor_copy(hts_bf[:], ht_ps[:])
                        if li == 3:
                            hts_f = lwork.tile([128, 128], F32, tag="hts_f")
                            nc.vector.tensor_copy(hts_f[:], ht_ps[:])
                            nc.sync.dma_start(hsh_d.ap(), hts_f[:])
                    nc.sync.dma_start(cc_h_in[li].ap(), hts_bf[:])
                    nc.gpsimd.collective_compute(
                        "AllGather", ALU.bypass, replica_groups=rgroups,
                        ins=[cc_h_in[li].ap().opt()],
                        outs=[cc_h_out[li].ap().opt()])
                    nc.sync.dma_start(
                        hT[:],
                        cc_h_out[li].ap().rearrange("(c p) b -> p c b", p=128))
                    if _DEBUG and li == 0:
                        dbgh = lwork.tile([128, PC * 128], F32, tag="dbgh")
                        nc.vector.tensor_copy(
                            dbgh[:].rearrange("p (c b) -> p c b", b=128),
                            hT[:])
                        nc.sync.dma_start(dbg_h1_d.ap(), dbgh[:])

                # ---- fc: logits = h @ fc_w + fc_b (natural output) ----
                lg_sb = lwork.tile([128, FCP], F32, tag="lg")
                with tc.tile_pool(name="psF", bufs=3, space="PSUM") as psF:
                    for nb in range(FCP // 512):
                        lg_ps = psF.tile([128, 512], F32, tag="lgp")
                        sl = slice(nb * 512, (nb + 1) * 512)
                        for k in range(PC):
                            nc.tensor.matmul(lg_ps[:], hT[:, k, :],
                                             fcw[:, k, sl],
                                             start=(k == 0), stop=False)
                        nc.tensor.matmul(lg_ps[:], ones128[:],
                                         fcb[0:1, sl], start=False, stop=True)
                        nc.scalar.copy(lg_sb[:, sl], lg_ps[:])
                        nc.sync.dma_start(logits_d.ap()[:, sl],
                                          lg_sb[:, sl])

    nc.compile()
    return nc


_NC_CACHE = None


def _get_nc():
    global _NC_CACHE
    if _NC_CACHE is None:
        _NC_CACHE = _build()
    return _NC_CACHE


def _prep_inputs(x, hidden, enc_output, W1_w, W1_b, W2_w, W2_b, V_w, V_b, emb,
                 l1_W, l1_b, l2_W, l2_b, l3_W, l3_b, l4_W, l4_b, fc_w, fc_b):
    """Host-side sharding/layout/casting. Returns in_maps for the 8 cores."""
    x = np.asarray(x).reshape(B).astype(np.int64)
    hidden = np.asarray(hidden, dtype=np.float32)
    enc_output = np.asarray(enc_output, dtype=np.float32)

    # shared (replicated) tensors
    w1 = _chunked(_to_bf16(W1_w))                  # [128, 8, U]
    w1b = _to_bf16(np.asarray(W1_b)).reshape(1, U)
    w2 = _chunked(_to_bf16(W2_w))
    w2bT = np.ascontiguousarray(
        np.asarray(W2_b, dtype=np.float32).reshape(PC, 128).T)
    vw = np.ascontiguousarray(
        _to_bf16(np.asarray(V_w).reshape(U)).reshape(PC, 128).T)
    # V_b shifts every score equally -> softmax-invariant; dropped.
    emb_bf = _to_bf16(emb)                          # [V, E]
    xw = np.ascontiguousarray(x.reshape(B, 1).astype(np.int32))
    fcb_full = np.asarray(fc_b, dtype=np.float32)

    lw_all, lb_all = [], []
    for W, bb in ((l1_W, l1_b), (l2_W, l2_b), (l3_W, l3_b), (l4_W, l4_b)):
        lw_all.append(_to_bf16(W))
        lb_all.append(np.asarray(bb, dtype=np.float32))

    in_maps = []
    for k in range(NC_N):
        bsl = slice(k * BS, (k + 1) * BS)
        enc_c = _to_bf16(enc_output[bsl]).reshape(BSS, U)
        encT = _chunked(np.ascontiguousarray(enc_c.T))      # [128, 8, 4096]
        hidT = _chunked(np.ascontiguousarray(
            _to_bf16(hidden[bsl]).T))                       # [128, 8, 16]

        m = {
            "encT": encT, "hidT": hidT,
            "w1": w1, "w1b": w1b, "w2": w2, "w2bT": w2bT, "vw": vw,
            "emb": emb_bf, "xidx": xw,
            "fcb": np.zeros((1, FCP), _BF),
        }
        m["fcb"][0, :FCC] = _to_bf16(fcb_full[k * FCC:(k + 1) * FCC])
        fcw_pad = np.zeros((U, FCP), _BF)
        fcw_pad[:, :FCC] = _to_bf16(
            np.asarray(fc_w)[:, k * FCC:(k + 1) * FCC])
        m["fcw"] = _chunked(fcw_pad)

        for i, (Wb, bb) in enumerate(zip(lw_all, lb_all), start=1):
            cols = np.concatenate(
                [Wb[:, g * U + k * GS:g * U + (k + 1) * GS]
                 for g in (0, 2, 3)], axis=1)               # i, g, o
            m[f"lw{i}"] = _chunked(np.ascontiguousarray(cols))
            m[f"lb{i}"] = np.ascontiguousarray(np.stack(
                [bb[g * U + k * GS:g * U + (k + 1) * GS]
                 for g in (0, 2, 3)], axis=1))              # [128, 3]
        in_maps.append(m)
    return in_maps


def _run(in_maps, trace=False):
    nc = _get_nc()
    return run_bass_kernel_spmd(
        nc, in_maps, core_ids=list(range(NC_N)), trace=trace)


def _assemble(results):
    logits = np.empty((B, V), np.float32)
    h = np.empty((B, U), np.float32)
    for k in range(NC_N):
        logits[:, k * FCC:(k + 1) * FCC] = results[k]["logits"][:, :FCC]
        h[:, k * GS:(k + 1) * GS] = results[k]["hsh"].T
    return logits, h


def kernel(**inputs):
    in_maps = _prep_inputs(**inputs)
    res = _run(in_maps, trace=False)
    return _assemble(res.results)


def kernel_traced(**inputs):
    """Like kernel() but with NTFF profiling; returns (outputs, exec_time_ns)."""
    in_maps = _prep_inputs(**inputs)
    res = _run(in_maps, trace=True)
    return _assemble(res.results), res.exec_time_ns


# revision 51
# speedup vs baseline: 1.1569x; 1.0197x over previous
"""Trainium2 Bass kernel: Bahdanau-attention decoder (attention + 4x LSTM + fc).

Contract: kernel(**inputs) takes the FULL unsharded inputs (as produced by
setup_inputs) and returns the full (logits, h) tuple, matching the reference.

Sharding (8 NeuronCores):
  - Attention is data-parallel over batch (16 rows/core). enc_output is staged
    host-side in transposed bf16 layout so the keys matmul contracts on the
    partition axis with no on-device transpose.
  - context is AllGathered (batch-major), then the 4 LSTM layers and the fc
    projection run tensor-parallel: each core owns a 128-wide slice of each
    LSTM gate and a 4000-wide slice of the vocab. h is kept transposed
    ([dims, batch]) so the AllGather's partition-axis concatenation rebuilds
    h^T directly. The forget gate is skipped entirely (c_prev == 0).
  - Matmul operands are bf16 (fp32 PE matmul runs at half rate); accumulation,
    softmax and gate math stays fp32.

Pipelining: the keys matmul runs column-block-major so each 512-column block
finishes all 8 u-chunks in sequence; its score matvec, exp (softmax without
the shift — scores are O(1), and softmax is shift-invariant so dropping the
max subtraction is exact) and the VectorE context reduction are interleaved
behind the PE stream. PSUM accumulation regions are always exactly one bank
(start=True clears has_written bank-wide).
"""

import numpy as np
import ml_dtypes

import concourse.bass as bass
import concourse.tile as tile
from concourse import bacc, mybir
from concourse.bass_utils import run_bass_kernel_spmd
from concourse.masks import make_identity

# Problem dims (hardcoded per the grading contract).
B, S, U, E, V = 128, 256, 1024, 256, 32000
NC_N = 8
BS = B // NC_N          # 16  batch rows per core
BSS = BS * S            # 4096 source positions per core
PC = U // 128           # 8   partition chunks of U
GS = U // NC_N          # 128 per-core slice width of each LSTM gate
FCC = V // NC_N         # 4000 true fc columns per core
FCP = 4096              # padded fc columns
CB = BSS // 512         # 8 column blocks (2 batch rows each)
CTXQ = 1                # column blocks per context batch

F32 = mybir.dt.float32
BF16 = mybir.dt.bfloat16
I32 = mybir.dt.int32
AF = mybir.ActivationFunctionType
ALU = mybir.AluOpType
AX = mybir.AxisListType

_BF = ml_dtypes.bfloat16
_DEBUG = False


def _to_bf16(x: np.ndarray) -> np.ndarray:
    """Fast fp32 -> bf16 with round-to-nearest-even (numpy bit trick)."""
    x = np.ascontiguousarray(x, dtype=np.float32)
    v = x.view(np.uint32)
    out = ((v + 0x7FFF + ((v >> 16) & 1)) >> 16).astype(np.uint16)
    return out.view(_BF)


def _chunked(a_bf16: np.ndarray) -> np.ndarray:
    """[C*128, N] -> [128, C, N] so sbuf[p, c, n] = a[c*128+p, n]."""
    cN, n = a_bf16.shape
    c = cN // 128
    return np.ascontiguousarray(
        a_bf16.reshape(c, 128, n).transpose(1, 0, 2)
    )


def _build():
    nc = bacc.Bacc("TRN2", target_bir_lowering=False, debug=False,
                   num_devices=NC_N)

    # ---- per-core external inputs ----
    encT_d = nc.dram_tensor("encT", [128, PC, BSS], BF16, kind="ExternalInput")
    hidT_d = nc.dram_tensor("hidT", [128, PC, BS], BF16, kind="ExternalInput")
    w1_d = nc.dram_tensor("w1", [128, PC, U], BF16, kind="ExternalInput")
    w1b_d = nc.dram_tensor("w1b", [1, U], BF16, kind="ExternalInput")
    w2_d = nc.dram_tensor("w2", [128, PC, U], BF16, kind="ExternalInput")
    w2bT_d = nc.dram_tensor("w2bT", [128, PC], F32, kind="ExternalInput")
    vw_d = nc.dram_tensor("vw", [128, PC], BF16, kind="ExternalInput")
    emb_d = nc.dram_tensor("emb", [V, E], BF16, kind="ExternalInput")
    xidx_d = nc.dram_tensor("xidx", [B, 1], I32, kind="ExternalInput")
    lw_d = [
        nc.dram_tensor(f"lw{i}", [128, kc, 3 * GS], BF16, kind="ExternalInput")
        for i, kc in enumerate((10, 16, 16, 16), start=1)
    ]
    lb_d = [
        nc.dram_tensor(f"lb{i}", [128, 3], F32, kind="ExternalInput")
        for i in range(1, 5)
    ]
    fcw_d = nc.dram_tensor("fcw", [128, PC, FCP], BF16, kind="ExternalInput")
    fcb_d = nc.dram_tensor("fcb", [1, FCP], BF16, kind="ExternalInput")

    # ---- per-core external outputs ----
    logits_d = nc.dram_tensor("logits", [B, FCP], F32, kind="ExternalOutput")
    hsh_d = nc.dram_tensor("hsh", [GS, B], F32, kind="ExternalOutput")
    if _DEBUG:
        dbg_q_d = nc.dram_tensor("dbg_q", [BS, U], F32, kind="ExternalOutput")
        dbg_ctx_d = nc.dram_tensor("dbg_ctx", [BS, U], F32,
                                   kind="ExternalOutput")
        dbg_h1_d = nc.dram_tensor("dbg_h1", [128, PC * 128], F32,
                                  kind="ExternalOutput")
        dbg_emb_d = nc.dram_tensor("dbg_emb", [128, (E // 128) * B], F32,
                                   kind="ExternalOutput")

    # ---- collective bounce buffers (bf16: payload feeds matmuls only) ----
    cc_warm_in = nc.dram_tensor("cc_warm_in", [1, 128], F32)
    cc_warm_out = nc.dram_tensor("cc_warm_out", [NC_N, 128], F32,
                                 addr_space="Shared")
    cc_ctx_in = nc.dram_tensor("cc_ctx_in", [BS, U], BF16)
    cc_ctx_out = nc.dram_tensor("cc_ctx_out", [B, U], BF16,
                                addr_space="Shared")
    cc_h_in = [nc.dram_tensor(f"cc_h_in{i}", [GS, B], BF16)
               for i in range(1, 5)]
    cc_h_out = [
        nc.dram_tensor(f"cc_h_out{i}", [U, B], BF16, addr_space="Shared")
        for i in range(1, 5)
    ]
    rgroups = [list(range(NC_N))]

    with tile.TileContext(nc) as tc:
        with tc.tile_pool(name="persist", bufs=1) as persist, \
             tc.tile_pool(name="work", bufs=1) as work:
            # ---- constants / small loads (issued in consumption order) ----
            id128 = persist.tile([128, 128], F32)
            make_identity(nc, id128[:])
            id128b = persist.tile([128, 128], BF16)
            make_identity(nc, id128b[:])
            ones16 = persist.tile([1, BS], BF16)
            nc.gpsimd.memset(ones16[:], 1.0)
            ones128 = persist.tile([1, 128], BF16)
            nc.gpsimd.memset(ones128[:], 1.0)

            # warm up the collective path while the big DMAs stream in
            warm_sb = persist.tile([1, 128], F32)
            nc.gpsimd.memset(warm_sb[:], 0.0)
            nc.sync.dma_start(cc_warm_in.ap(), warm_sb[:])
            nc.gpsimd.collective_compute(
                "AllGather", ALU.bypass, replica_groups=rgroups,
                ins=[cc_warm_in.ap().opt()], outs=[cc_warm_out.ap().opt()])

            hidT = persist.tile([128, PC, BS], BF16)
            nc.sync.dma_start(hidT[:], hidT_d.ap())
            w2bT = persist.tile([128, PC], F32)
            nc.sync.dma_start(w2bT[:], w2bT_d.ap())
            vw = persist.tile([128, PC], BF16)
            nc.sync.dma_start(vw[:], vw_d.ap())
            w1b = persist.tile([1, U], BF16)
            nc.sync.dma_start(w1b[:], w1b_d.ap())
            xidx = persist.tile([B, 1], I32)
            nc.sync.dma_start(xidx[:], xidx_d.ap())

            # long-lived transposed activations (bf16 matmul operands)
            qTb = persist.tile([128, PC, BS], F32)      # q^T + W2_b
            ctxTu = persist.tile([128, PC, BS], F32)    # unnormalized ctx^T
            ctxT = persist.tile([128, PC, 128], BF16)   # full context^T
            hT = persist.tile([128, PC, 128], BF16, tag="hT")  # full h^T
            embT = persist.tile([128, E // 128, B], BF16)

            # attention working tiles
            p_row = work.tile([1, BSS], BF16)           # exp(score), unnorm
            ssum_row = work.tile([1, BS], F32)
            ssumP = work.tile([BS, 1], F32)
            rsum = work.tile([BS, 1], F32)
            p_bc = work.tile([128, BSS], BF16)
            ctx_nat = work.tile([BS, U], F32)
            ctx_nat_bf = work.tile([BS, U], BF16)
            lbs = []
            for i in range(4):
                lb = persist.tile([128, 3], F32, tag=f"lb{i}")
                nc.sync.dma_start(lb[:], lb_d[i].ap())
                lbs.append(lb)
            fcb = persist.tile([1, FCP], BF16)
            nc.sync.dma_start(fcb[:], fcb_d.ap())
            # LSTM weights up-front: loading them after attention would
            # flood HBM exactly when the ctx/h1 AllGathers need it
            lws = []
            for i, kc in enumerate((10, 16, 16, 16)):
                lw = persist.tile([128, kc, 3 * GS], BF16, tag=f"lw{i}",
                                  name=f"lwt{i}")
                for k in range(kc):
                    nc.sync.dma_start(lw[:, k, :], lw_d[i].ap()[:, k, :])
                lws.append(lw)

            with tc.tile_pool(name="bigA", bufs=1) as bigA:
                # w1/w2 first: q (hence the tanh bias) gates the keys
                # eviction pipeline, so it must not sit behind the 8 MB encT.
                w1 = bigA.tile([128, PC, U], BF16)
                for c in range(PC):
                    nc.sync.dma_start(w1[:, c, :], w1_d.ap()[:, c, :])
                w2 = bigA.tile([128, PC, U], BF16)
                for c in range(PC):
                    nc.sync.dma_start(w2[:, c, :], w2_d.ap()[:, c, :])
                encT = bigA.tile([128, PC, BSS], BF16)
                for cb in range(CB):
                    sl = slice(cb * 512, (cb + 1) * 512)
                    nc.sync.dma_start(encT[:, :, sl], encT_d.ap()[:, :, sl])

                # embedding gather (natural) + PE transpose ->
                # embT[p, c, i] = emb[x_i, c*128+p]
                emb_nat = bigA.tile([B, E], BF16)
                nc.gpsimd.indirect_dma_start(
                    emb_nat[:], None, emb_d.ap(),
                    bass.IndirectOffsetOnAxis(ap=xidx[:], axis=0))

                # ---- phase A: q = hidden @ W1 + W1_b (natural [16, U]) ----
                with tc.tile_pool(name="psA", bufs=1, space="PSUM") as psA, \
                     tc.tile_pool(name="sbA", bufs=1) as sbA:
                    q_ps = psA.tile([BS, U], F32)
                    for nb in range(2):
                        sl = slice(nb * 512, (nb + 1) * 512)
                        for k in range(PC):
                            nc.tensor.matmul(
                                q_ps[:, sl], hidT[:, k, :], w1[:, k, sl],
                                start=(k == 0), stop=False)
                        nc.tensor.matmul(
                            q_ps[:, sl], ones16[:], w1b[0:1, sl],
                            start=False, stop=True)
                    q_sb = sbA.tile([BS, U], F32)
                    nc.scalar.copy(q_sb[:], q_ps[:])
                    if _DEBUG:
                        nc.sync.dma_start(dbg_q_d.ap(), q_sb[:])
                    # qT chunks (+ W2_b per-partition) for the tanh bias
                    for c in range(PC):
                        qt_ps = psA.tile([128, BS], F32, tag="qt")
                        nc.tensor.matmul(
                            qt_ps[:], q_sb[:, c * 128:(c + 1) * 128],
                            id128[0:BS, 0:BS], is_transpose=True)
                        nc.vector.tensor_add(
                            qTb[:, c, :], qt_ps[:],
                            w2bT[:, c:c + 1].to_broadcast((128, BS)))
                    # emb transpose rides in this psum pool too
                    for c in range(E // 128):
                        ep = psA.tile([128, B], BF16, tag="ep")
                        nc.tensor.matmul(
                            ep[:], emb_nat[:, c * 128:(c + 1) * 128],
                            id128b[:], is_transpose=True)
                        nc.vector.tensor_copy(embT[:, c, :], ep[:])
                if _DEBUG:
                    dbge = work.tile([128, (E // 128) * B], F32, tag="dbge")
                    nc.vector.tensor_copy(
                        dbge[:].rearrange("p (c b) -> p c b", b=B), embT[:])
                    nc.sync.dma_start(dbg_emb_d.ap(), dbge[:])

                # ---- fused keys -> tanh -> score -> exp -> context ----
                # column-block-major: each 512-col block (2 batch rows)
                # completes keys for all 8 u-chunks, then its score matvec
                # and exp run while the PE streams the next block. Context
                # (VectorE) runs per CTXQ blocks, hidden under the PE.
                with tc.tile_pool(name="psK", bufs=4, space="PSUM") as psK, \
                     tc.tile_pool(name="psS", bufs=2, space="PSUM") as psS, \
                     tc.tile_pool(name="psBC", bufs=2, space="PSUM") as psBC, \
                     tc.tile_pool(name="tanhp", bufs=2) as tanhp, \
                     tc.tile_pool(name="prodp", bufs=2) as prodp:
                    for cb in range(CB):
                        csl = slice(cb * 512, (cb + 1) * 512)
                        th = tanhp.tile([128, PC, 512], BF16, tag="th")
                        for m in range(PC):
                            kp = psK.tile([128, 512], F32, tag="kp")
                            for k in range(PC):
                                nc.tensor.matmul(
                                    kp[:], w2[:, k, m * 128:(m + 1) * 128],
                                    encT[:, k, csl],
                                    start=(k == 0), stop=(k == PC - 1))
                            for o in range(2):
                                b = cb * 2 + o
                                nc.scalar.activation(
                                    th[:, m, o * S:(o + 1) * S],
                                    kp[:, o * S:(o + 1) * S],
                                    AF.Tanh, bias=qTb[:, m, b:b + 1])
                        # score for this block
                        sp = psS.tile([1, 512], F32, tag="sp")
                        for k in range(PC):
                            nc.tensor.matmul(
                                sp[:], vw[:, k:k + 1], th[:, k, :],
                                start=(k == 0), stop=(k == PC - 1))
                        # exp (softmax shift dropped: shift-invariant) + sums
                        for o in range(2):
                            b = cb * 2 + o
                            nc.scalar.activation(
                                p_row[0:1, b * S:(b + 1) * S],
                                sp[0:1, o * S:(o + 1) * S], AF.Exp)
                        nc.vector.reduce_sum(
                            out=ssum_row[0:1, cb * 2:cb * 2 + 2],
                            in_=p_row[0:1, csl].rearrange(
                                "p (b s) -> p b s", s=S),
                            axis=AX.X)
                        # broadcast exp row across partitions (PE rank-1)
                        bc_ps = psBC.tile([128, 512], F32, tag="bc")
                        nc.tensor.matmul(bc_ps[:], ones128[:], p_row[0:1, csl])
                        nc.vector.tensor_copy(p_bc[:, csl], bc_ps[:])
                        # context contribution for the finished quarter
                        if (cb + 1) % CTXQ == 0:
                            qsl = slice((cb + 1 - CTXQ) * 512, (cb + 1) * 512)
                            bsl = slice((cb + 1 - CTXQ) * 2, (cb + 1) * 2)
                            for c in range(PC):
                                prod = prodp.tile([128, CTXQ * 512], BF16,
                                                  tag="prod")
                                nc.vector.tensor_mul(prod[:], encT[:, c, qsl],
                                                     p_bc[:, qsl])
                                nc.vector.reduce_sum(
                                    out=ctxTu[:, c, bsl],
                                    in_=prod[:].rearrange(
                                        "p (b s) -> p b s", s=S),
                                    axis=AX.X)

                # ---- normalize context + to natural layout + AllGather ----
                nc.sync.dma_start(ssumP[:], ssum_row[0:1, :])
                nc.vector.reciprocal(rsum[:], ssumP[:])
                with tc.tile_pool(name="psC", bufs=2, space="PSUM") as psC:
                    for c in range(PC):
                        cn_ps = psC.tile([BS, 128], F32, tag="cn")
                        nc.tensor.matmul(cn_ps[:], ctxTu[:, c, :],
                                         id128[:], is_transpose=True)
                        nc.vector.tensor_scalar_mul(
                            ctx_nat[:, c * 128:(c + 1) * 128], cn_ps[:],
                            rsum[:])
                    nc.vector.tensor_copy(ctx_nat_bf[:], ctx_nat[:])
                    if _DEBUG:
                        nc.sync.dma_start(dbg_ctx_d.ap(), ctx_nat[:])
                    nc.sync.dma_start(cc_ctx_in.ap(), ctx_nat_bf[:])
                    nc.gpsimd.collective_compute(
                        "AllGather", ALU.bypass, replica_groups=rgroups,
                        ins=[cc_ctx_in.ap().opt()],
                        outs=[cc_ctx_out.ap().opt()])
                    ctxn = work.tile([128, U], BF16)
                    nc.sync.dma_start(ctxn[:], cc_ctx_out.ap())
                    for c in range(PC):
                        ct_ps = psC.tile([128, 128], BF16, tag="ct")
                        nc.tensor.matmul(ct_ps[:],
                                         ctxn[:, c * 128:(c + 1) * 128],
                                         id128b[:], is_transpose=True)
                        nc.vector.tensor_copy(ctxT[:, c, :], ct_ps[:])

            # ---- LSTM + fc phase (bigA space reused) ----
            with tc.tile_pool(name="bigB", bufs=1) as bigB, \
                 tc.tile_pool(name="lwork", bufs=1) as lwork:
                fcw = bigB.tile([128, PC, FCP], BF16)
                for c in range(PC):
                    nc.sync.dma_start(fcw[:, c, :], fcw_d.ap()[:, c, :])

                for li in range(4):
                    kc = 10 if li == 0 else 16
                    xtail = embT if li == 0 else hT
                    with tc.tile_pool(name=f"psL{li}", bufs=1,
                                      space="PSUM") as psL:
                        z_ps = psL.tile([128, 3 * GS], F32, tag="z")
                        for k in range(kc):
                            lhsT = (ctxT[:, k, :] if k < PC
                                    else xtail[:, k - PC, :])
                            nc.tensor.matmul(z_ps[:], lhsT, lws[li][:, k, :],
                                             start=(k == 0),
                                             stop=(k == kc - 1))
                        i_s = lwork.tile([128, GS], F32, tag="i_s")
                        g_t = lwork.tile([128, GS], F32, tag="g_t")
                        o_s = lwork.tile([128, GS], F32, tag="o_s")
                        c_f = lwork.tile([128, GS], F32, tag="c_f")
                        c_t = lwork.tile([128, GS], F32, tag="c_t")
                        h_n = lwork.tile([128, GS], F32, tag="h_n")
                        nc.scalar.activation(i_s[:], z_ps[:, 0:GS],
                                             AF.Sigmoid,
                                             bias=lbs[li][:, 0:1])
                        nc.scalar.activation(g_t[:], z_ps[:, GS:2 * GS],
                                             AF.Tanh, bias=lbs[li][:, 1:2])
                        nc.scalar.activation(o_s[:], z_ps[:, 2 * GS:3 * GS],
                                             AF.Sigmoid,
                                             bias=lbs[li][:, 2:3])
                        nc.vector.tensor_mul(c_f[:], i_s[:], g_t[:])
                        nc.scalar.activation(c_t[:], c_f[:], AF.Tanh)
                        nc.vector.tensor_mul(h_n[:], o_s[:], c_t[:])
                        # h^T shard for the partition-axis AllGather
                        ht_ps = psL.tile([128, 128], F32, tag="ht")
                        nc.tensor.matmul(ht_ps[:], h_n[:], id128[:],
                                         is_transpose=True)
                        hts_bf = lwork.tile([128, 128], BF16, tag="hts_bf")
                        nc.vector.tensor_copy(hts_bf[:], ht_ps[:])
                        if li == 3:
                            hts_f = lwork.tile([128, 128], F32, tag="hts_f")
                            nc.vector.tensor_copy(hts_f[:], ht_ps[:])
                            nc.sync.dma_start(hsh_d.ap(), hts_f[:])
                    nc.sync.dma_start(cc_h_in[li].ap(), hts_bf[:])
                    nc.gpsimd.collective_compute(
                        "AllGather", ALU.bypass, replica_groups=rgroups,
                        ins=[cc_h_in[li].ap().opt()],
                        outs=[cc_h_out[li].ap().opt()])
                    nc.sync.dma_start(
                        hT[:],
                        cc_h_out[li].ap().rearrange("(c p) b -> p c b", p=128))
                    if _DEBUG and li == 0:
                        dbgh = lwork.tile([128, PC * 128], F32, tag="dbgh")
                        nc.vector.tensor_copy(
                            dbgh[:].rearrange("p (c b) -> p c b", b=128),
                            hT[:])
                        nc.sync.dma_start(dbg_h1_d.ap(), dbgh[:])

                # ---- fc: logits = h @ fc_w + fc_b (natural output) ----
                lg_sb = lwork.tile([128, FCP], F32, tag="lg")
                with tc.tile_pool(name="psF", bufs=3, space="PSUM") as psF:
                    for nb in range(FCP // 512):
                        lg_ps = psF.tile([128, 512], F32, tag="lgp")
                        sl = slice(nb * 512, (nb + 1) * 512)
                        for k in range(PC):
                            nc.tensor.matmul(lg_ps[:], hT[:, k, :],
                                             fcw[:, k, sl],
                                             start=(k == 0), stop=False)
                        nc.tensor.matmul(lg_ps[:], ones128[:],
                                         fcb[0:1, sl], start=False, stop=True)
                        nc.scalar.copy(lg_sb[:, sl], lg_ps[:])
                        nc.sync.dma_start(logits_d.ap()[:, sl],
                                          lg_sb[:, sl])

    nc.compile()
    return nc


_NC_CACHE = None


def _get_nc():
    global _NC_CACHE
    if _NC_CACHE is None:
        _NC_CACHE = _build()
    return _NC_CACHE


def _prep_inputs(x, hidden, enc_output, W1_w, W1_b, W2_w, W2_b, V_w, V_b, emb,
                 l1_W, l1_b, l2_W, l2_b, l3_W, l3_b, l4_W, l4_b, fc_w, fc_b):
    """Host-side sharding/layout/casting. Returns in_maps for the 8 cores."""
    x = np.asarray(x).reshape(B).astype(np.int64)
    hidden = np.asarray(hidden, dtype=np.float32)
    enc_output = np.asarray(enc_output, dtype=np.float32)

    # shared (replicated) tensors
    w1 = _chunked(_to_bf16(W1_w))                  # [128, 8, U]
    w1b = _to_bf16(np.asarray(W1_b)).reshape(1, U)
    w2 = _chunked(_to_bf16(W2_w))
    w2bT = np.ascontiguousarray(
        np.asarray(W2_b, dtype=np.float32).reshape(PC, 128).T)
    vw = np.ascontiguousarray(
        _to_bf16(np.asarray(V_w).reshape(U)).reshape(PC, 128).T)
    # V_b shifts every score equally -> softmax-invariant; dropped.
    emb_bf = _to_bf16(emb)                          # [V, E]
    xw = np.ascontiguousarray(x.reshape(B, 1).astype(np.int32))
    fcb_full = np.asarray(fc_b, dtype=np.float32)

    lw_all, lb_all = [], []
    for W, bb in ((l1_W, l1_b), (l2_W, l2_b), (l3_W, l3_b), (l4_W, l4_b)):
        lw_all.append(_to_bf16(W))
        lb_all.append(np.asarray(bb, dtype=np.float32))

    in_maps = []
    for k in range(NC_N):
        bsl = slice(k * BS, (k + 1) * BS)
        enc_c = _to_bf16(enc_output[bsl]).reshape(BSS, U)
        encT = _chunked(np.ascontiguousarray(enc_c.T))      # [128, 8, 4096]
        hidT = _chunked(np.ascontiguousarray(
            _to_bf16(hidden[bsl]).T))                       # [128, 8, 16]

        m = {
            "encT": encT, "hidT": hidT,
            "w1": w1, "w1b": w1b, "w2": w2, "w2bT": w2bT, "vw": vw,
            "emb": emb_bf, "xidx": xw,
            "fcb": np.zeros((1, FCP), _BF),
        }
        m["fcb"][0, :FCC] = _to_bf16(fcb_full[k * FCC:(k + 1) * FCC])
        fcw_pad = np.zeros((U, FCP), _BF)
        fcw_pad[:, :FCC] = _to_bf16(
            np.asarray(fc_w)[:, k * FCC:(k + 1) * FCC])
        m["fcw"] = _chunked(fcw_pad)

        for i, (Wb, bb) in enumerate(zip(lw_all, lb_all), start=1):
            cols = np.concatenate(
                [Wb[:, g * U + k * GS:g * U + (k + 1) * GS]
                 for g in (0, 2, 3)], axis=1)               # i, g, o
            m[f"lw{i}"] = _chunked(np.ascontiguousarray(cols))
            m[f"lb{i}"] = np.ascontiguousarray(np.stack(
                [bb[g * U + k * GS:g * U + (k + 1) * GS]
                 for g in (0, 2, 3)], axis=1))              # [128, 3]
        in_maps.append(m)
    return in_maps


def _run(in_maps, trace=False):
    nc = _get_nc()
    return run_bass_kernel_spmd(
        nc, in_maps, core_ids=list(range(NC_N)), trace=trace)


def _assemble(results):
    logits = np.empty((B, V), np.float32)
    h = np.empty((B, U), np.float32)
    for k in range(NC_N):
        logits[:, k * FCC:(k + 1) * FCC] = results[k]["logits"][:, :FCC]
        h[:, k * GS:(k + 1) * GS] = results[k]["hsh"].T
    return logits, h


def kernel(**inputs):
    in_maps = _prep_inputs(**inputs)
    res = _run(in_maps, trace=False)
    return _assemble(res.results)


def kernel_traced(**inputs):
    """Like kernel() but with NTFF profiling; returns (outputs, exec_time_ns)."""
    in_maps = _prep_inputs(**inputs)
    res = _run(in_maps, trace=True)
    return _assemble(res.results), res.exec_time_ns


# revision 52
# speedup vs baseline: 1.2446x; 1.0758x over previous
"""Trainium2 Bass kernel: Bahdanau-attention decoder (attention + 4x LSTM + fc).

Contract: kernel(**inputs) takes the FULL unsharded inputs (as produced by
setup_inputs) and returns the full (logits, h) tuple, matching the reference.

Sharding (8 NeuronCores):
  - Attention is data-parallel over batch (16 rows/core). enc_output is staged
    host-side in transposed bf16 layout so the keys matmul contracts on the
    partition axis with no on-device transpose.
  - context is AllGathered (batch-major), then the 4 LSTM layers and the fc
    projection run tensor-parallel: each core owns a 128-wide slice of each
    LSTM gate and a 4000-wide slice of the vocab. h is kept transposed
    ([dims, batch]) so the AllGather's partition-axis concatenation rebuilds
    h^T directly. The forget gate is skipped entirely (c_prev == 0).
  - Matmul operands are bf16 (fp32 PE matmul runs at half rate); accumulation,
    softmax and gate math stays fp32.

Pipelining: the keys matmul runs column-block-major so each 512-column block
finishes all 8 u-chunks in sequence; its score matvec, exp (softmax without
the shift — scores are O(1), and softmax is shift-invariant so dropping the
max subtraction is exact) and the VectorE context reduction are interleaved
behind the PE stream. PSUM accumulation regions are always exactly one bank
(start=True clears has_written bank-wide).
"""

import numpy as np
import ml_dtypes

import concourse.bass as bass
import concourse.tile as tile
from concourse import bacc, mybir
from concourse.bass_utils import run_bass_kernel_spmd
from concourse.masks import make_identity

# Problem dims (hardcoded per the grading contract).
B, S, U, E, V = 128, 256, 1024, 256, 32000
NC_N = 8
BS = B // NC_N          # 16  batch rows per core
BSS = BS * S            # 4096 source positions per core
PC = U // 128           # 8   partition chunks of U
GS = U // NC_N          # 128 per-core slice width of each LSTM gate
FCC = V // NC_N         # 4000 true fc columns per core
FCP = 4096              # padded fc columns
CB = BSS // 512         # 8 column blocks (2 batch rows each)
CTXQ = 1                # column blocks per context batch

F32 = mybir.dt.float32
BF16 = mybir.dt.bfloat16
I32 = mybir.dt.int32
AF = mybir.ActivationFunctionType
ALU = mybir.AluOpType
AX = mybir.AxisListType

_BF = ml_dtypes.bfloat16
_DEBUG = False


def _to_bf16(x: np.ndarray) -> np.ndarray:
    """Fast fp32 -> bf16 with round-to-nearest-even (numpy bit trick)."""
    x = np.ascontiguousarray(x, dtype=np.float32)
    v = x.view(np.uint32)
    out = ((v + 0x7FFF + ((v >> 16) & 1)) >> 16).astype(np.uint16)
    return out.view(_BF)


def _chunked(a_bf16: np.ndarray) -> np.ndarray:
    """[C*128, N] -> [128, C, N] so sbuf[p, c, n] = a[c*128+p, n]."""
    cN, n = a_bf16.shape
    c = cN // 128
    return np.ascontiguousarray(
        a_bf16.reshape(c, 128, n).transpose(1, 0, 2)
    )


def _build():
    nc = bacc.Bacc("TRN2", target_bir_lowering=False, debug=False,
                   num_devices=NC_N)

    # ---- per-core external inputs ----
    encT_d = nc.dram_tensor("encT", [128, PC, BSS], BF16, kind="ExternalInput")
    hidT_d = nc.dram_tensor("hidT", [128, PC, BS], BF16, kind="ExternalInput")
    w1_d = nc.dram_tensor("w1", [128, PC, U], BF16, kind="ExternalInput")
    w1b_d = nc.dram_tensor("w1b", [1, U], BF16, kind="ExternalInput")
    w2_d = nc.dram_tensor("w2", [128, PC, U], BF16, kind="ExternalInput")
    w2bT_d = nc.dram_tensor("w2bT", [128, PC], F32, kind="ExternalInput")
    vw_d = nc.dram_tensor("vw", [128, PC], BF16, kind="ExternalInput")
    emb_d = nc.dram_tensor("emb", [V, E], BF16, kind="ExternalInput")
    xidx_d = nc.dram_tensor("xidx", [B, 1], I32, kind="ExternalInput")
    lw_d = [
        nc.dram_tensor(f"lw{i}", [128, kc, 3 * GS], BF16, kind="ExternalInput")
        for i, kc in enumerate((10, 16, 16, 16), start=1)
    ]
    lb_d = [
        nc.dram_tensor(f"lb{i}", [128, 3], F32, kind="ExternalInput")
        for i in range(1, 5)
    ]
    fcw_d = nc.dram_tensor("fcw", [128, PC, FCP], BF16, kind="ExternalInput")
    fcb_d = nc.dram_tensor("fcb", [1, FCP], BF16, kind="ExternalInput")

    # ---- per-core external outputs ----
    logits_d = nc.dram_tensor("logits", [B, FCP], F32, kind="ExternalOutput")
    hsh_d = nc.dram_tensor("hsh", [GS, B], F32, kind="ExternalOutput")
    if _DEBUG:
        dbg_q_d = nc.dram_tensor("dbg_q", [BS, U], F32, kind="ExternalOutput")
        dbg_ctx_d = nc.dram_tensor("dbg_ctx", [BS, U], F32,
                                   kind="ExternalOutput")
        dbg_h1_d = nc.dram_tensor("dbg_h1", [128, PC * 128], F32,
                                  kind="ExternalOutput")
        dbg_emb_d = nc.dram_tensor("dbg_emb", [128, (E // 128) * B], F32,
                                   kind="ExternalOutput")

    # ---- collective bounce buffers (bf16: payload feeds matmuls only) ----
    cc_warm_in = nc.dram_tensor("cc_warm_in", [1, 128], F32)
    cc_warm_out = nc.dram_tensor("cc_warm_out", [NC_N, 128], F32,
                                 addr_space="Shared")
    cc_ctx_in = nc.dram_tensor("cc_ctx_in", [BS, U], BF16)
    cc_ctx_out = nc.dram_tensor("cc_ctx_out", [B, U], BF16,
                                addr_space="Shared")
    cc_h_in = [nc.dram_tensor(f"cc_h_in{i}", [GS, B], BF16)
               for i in range(1, 5)]
    cc_h_out = [
        nc.dram_tensor(f"cc_h_out{i}", [U, B], BF16, addr_space="Shared")
        for i in range(1, 5)
    ]
    rgroups = [list(range(NC_N))]

    with tile.TileContext(nc) as tc:
        with tc.tile_pool(name="persist", bufs=1) as persist, \
             tc.tile_pool(name="work", bufs=1) as work:
            # ---- constants / small loads (issued in consumption order) ----
            id128 = persist.tile([128, 128], F32)
            make_identity(nc, id128[:])
            id128b = persist.tile([128, 128], BF16)
            make_identity(nc, id128b[:])
            ones16 = persist.tile([1, BS], BF16)
            nc.gpsimd.memset(ones16[:], 1.0)
            ones128 = persist.tile([1, 128], BF16)
            nc.gpsimd.memset(ones128[:], 1.0)

            # warm up the collective path while the big DMAs stream in
            warm_sb = persist.tile([1, 128], F32)
            nc.gpsimd.memset(warm_sb[:], 0.0)
            nc.sync.dma_start(cc_warm_in.ap(), warm_sb[:])
            nc.gpsimd.collective_compute(
                "AllGather", ALU.bypass, replica_groups=rgroups,
                ins=[cc_warm_in.ap().opt()], outs=[cc_warm_out.ap().opt()])

            hidT = persist.tile([128, PC, BS], BF16)
            nc.sync.dma_start(hidT[:], hidT_d.ap())
            w2bT = persist.tile([128, PC], F32)
            nc.sync.dma_start(w2bT[:], w2bT_d.ap())
            vw = persist.tile([128, PC], BF16)
            nc.sync.dma_start(vw[:], vw_d.ap())
            w1b = persist.tile([1, U], BF16)
            nc.sync.dma_start(w1b[:], w1b_d.ap())
            xidx = persist.tile([B, 1], I32)
            nc.sync.dma_start(xidx[:], xidx_d.ap())

            # long-lived transposed activations (bf16 matmul operands)
            qTb = persist.tile([128, PC, BS], F32)      # q^T + W2_b
            ctxTu = persist.tile([128, PC, BS], F32)    # unnormalized ctx^T
            ctxT = persist.tile([128, PC, 128], BF16)   # full context^T
            hT = persist.tile([128, PC, 128], BF16, tag="hT")  # full h^T
            embT = persist.tile([128, E // 128, B], BF16)

            # attention working tiles
            p_row = work.tile([1, BSS], BF16)           # exp(score), unnorm
            ssum_row = work.tile([1, BS], F32)
            ssumP = work.tile([BS, 1], F32)
            rsum = work.tile([BS, 1], F32)
            p_bc = work.tile([128, BSS], BF16)
            ctx_nat = work.tile([BS, U], F32)
            ctx_nat_bf = work.tile([BS, U], BF16)
            lbs = []
            for i in range(4):
                lb = persist.tile([128, 3], F32, tag=f"lb{i}")
                nc.sync.dma_start(lb[:], lb_d[i].ap())
                lbs.append(lb)
            fcb = persist.tile([1, FCP], BF16)
            nc.sync.dma_start(fcb[:], fcb_d.ap())
            # LSTM weights live in early SBUF (so their loads can overlap
            # attention instead of flooding HBM at the ctx AllGather), but
            # the dma_starts are issued after encT's so startup stays fast
            lws = [persist.tile([128, kc, 3 * GS], BF16, tag=f"lw{i}",
                                name=f"lwt{i}")
                   for i, kc in enumerate((10, 16, 16, 16))]

            with tc.tile_pool(name="bigA", bufs=1) as bigA:
                # w1/w2 first: q (hence the tanh bias) gates the keys
                # eviction pipeline, so it must not sit behind the 8 MB encT.
                w1 = bigA.tile([128, PC, U], BF16)
                for c in range(PC):
                    nc.sync.dma_start(w1[:, c, :], w1_d.ap()[:, c, :])
                w2 = bigA.tile([128, PC, U], BF16)
                for c in range(PC):
                    nc.sync.dma_start(w2[:, c, :], w2_d.ap()[:, c, :])
                encT = bigA.tile([128, PC, BSS], BF16)
                for cb in range(CB):
                    sl = slice(cb * 512, (cb + 1) * 512)
                    nc.sync.dma_start(encT[:, :, sl], encT_d.ap()[:, :, sl])

                for i, kc in enumerate((10, 16, 16, 16)):
                    for k in range(kc):
                        nc.sync.dma_start(lws[i][:, k, :],
                                          lw_d[i].ap()[:, k, :])

                # embedding gather (natural) + PE transpose ->
                # embT[p, c, i] = emb[x_i, c*128+p]
                emb_nat = bigA.tile([B, E], BF16)
                nc.gpsimd.indirect_dma_start(
                    emb_nat[:], None, emb_d.ap(),
                    bass.IndirectOffsetOnAxis(ap=xidx[:], axis=0))

                # ---- phase A: q = hidden @ W1 + W1_b (natural [16, U]) ----
                with tc.tile_pool(name="psA", bufs=1, space="PSUM") as psA, \
                     tc.tile_pool(name="sbA", bufs=1) as sbA:
                    q_ps = psA.tile([BS, U], F32)
                    for nb in range(2):
                        sl = slice(nb * 512, (nb + 1) * 512)
                        for k in range(PC):
                            nc.tensor.matmul(
                                q_ps[:, sl], hidT[:, k, :], w1[:, k, sl],
                                start=(k == 0), stop=False)
                        nc.tensor.matmul(
                            q_ps[:, sl], ones16[:], w1b[0:1, sl],
                            start=False, stop=True)
                    q_sb = sbA.tile([BS, U], F32)
                    nc.scalar.copy(q_sb[:], q_ps[:])
                    if _DEBUG:
                        nc.sync.dma_start(dbg_q_d.ap(), q_sb[:])
                    # qT chunks (+ W2_b per-partition) for the tanh bias
                    for c in range(PC):
                        qt_ps = psA.tile([128, BS], F32, tag="qt")
                        nc.tensor.matmul(
                            qt_ps[:], q_sb[:, c * 128:(c + 1) * 128],
                            id128[0:BS, 0:BS], is_transpose=True)
                        nc.vector.tensor_add(
                            qTb[:, c, :], qt_ps[:],
                            w2bT[:, c:c + 1].to_broadcast((128, BS)))
                    # emb transpose rides in this psum pool too
                    for c in range(E // 128):
                        ep = psA.tile([128, B], BF16, tag="ep")
                        nc.tensor.matmul(
                            ep[:], emb_nat[:, c * 128:(c + 1) * 128],
                            id128b[:], is_transpose=True)
                        nc.vector.tensor_copy(embT[:, c, :], ep[:])
                if _DEBUG:
                    dbge = work.tile([128, (E // 128) * B], F32, tag="dbge")
                    nc.vector.tensor_copy(
                        dbge[:].rearrange("p (c b) -> p c b", b=B), embT[:])
                    nc.sync.dma_start(dbg_emb_d.ap(), dbge[:])

                # ---- fused keys -> tanh -> score -> exp -> context ----
                # column-block-major: each 512-col block (2 batch rows)
                # completes keys for all 8 u-chunks, then its score matvec
                # and exp run while the PE streams the next block. Context
                # (VectorE) runs per CTXQ blocks, hidden under the PE.
                with tc.tile_pool(name="psK", bufs=4, space="PSUM") as psK, \
                     tc.tile_pool(name="psS", bufs=2, space="PSUM") as psS, \
                     tc.tile_pool(name="tanhp", bufs=2) as tanhp, \
                     tc.tile_pool(name="prodp", bufs=2) as prodp:
                    for cb in range(CB):
                        csl = slice(cb * 512, (cb + 1) * 512)
                        th = tanhp.tile([128, PC, 512], BF16, tag="th")
                        for m in range(PC):
                            kp = psK.tile([128, 512], F32, tag="kp")
                            for k in range(PC):
                                nc.tensor.matmul(
                                    kp[:], w2[:, k, m * 128:(m + 1) * 128],
                                    encT[:, k, csl],
                                    start=(k == 0), stop=(k == PC - 1))
                            for o in range(2):
                                b = cb * 2 + o
                                nc.scalar.activation(
                                    th[:, m, o * S:(o + 1) * S],
                                    kp[:, o * S:(o + 1) * S],
                                    AF.Tanh, bias=qTb[:, m, b:b + 1])
                        # score for this block
                        sp = psS.tile([1, 512], F32, tag="sp")
                        for k in range(PC):
                            nc.tensor.matmul(
                                sp[:], vw[:, k:k + 1], th[:, k, :],
                                start=(k == 0), stop=(k == PC - 1))
                        # exp (softmax shift dropped: shift-invariant) + sums
                        for o in range(2):
                            b = cb * 2 + o
                            nc.scalar.activation(
                                p_row[0:1, b * S:(b + 1) * S],
                                sp[0:1, o * S:(o + 1) * S], AF.Exp)
                        nc.vector.reduce_sum(
                            out=ssum_row[0:1, cb * 2:cb * 2 + 2],
                            in_=p_row[0:1, csl].rearrange(
                                "p (b s) -> p b s", s=S),
                            axis=AX.X)
                        # broadcast exp row across partitions (GpSimd)
                        nc.gpsimd.partition_broadcast(
                            p_bc[:, csl], p_row[0:1, csl])
                        # context contribution for the finished quarter
                        if (cb + 1) % CTXQ == 0:
                            qsl = slice((cb + 1 - CTXQ) * 512, (cb + 1) * 512)
                            bsl = slice((cb + 1 - CTXQ) * 2, (cb + 1) * 2)
                            for c in range(PC):
                                prod = prodp.tile([128, CTXQ * 512], BF16,
                                                  tag="prod")
                                nc.vector.tensor_mul(prod[:], encT[:, c, qsl],
                                                     p_bc[:, qsl])
                                nc.vector.reduce_sum(
                                    out=ctxTu[:, c, bsl],
                                    in_=prod[:].rearrange(
                                        "p (b s) -> p b s", s=S),
                                    axis=AX.X)

                # ---- normalize context + to natural layout + AllGather ----
                nc.sync.dma_start(ssumP[:], ssum_row[0:1, :])
                nc.vector.reciprocal(rsum[:], ssumP[:])
                with tc.tile_pool(name="psC", bufs=2, space="PSUM") as psC:
                    for c in range(PC):
                        cn_ps = psC.tile([BS, 128], F32, tag="cn")
                        nc.tensor.matmul(cn_ps[:], ctxTu[:, c, :],
                                         id128[:], is_transpose=True)
                        nc.vector.tensor_scalar_mul(
                            ctx_nat[:, c * 128:(c + 1) * 128], cn_ps[:],
                            rsum[:])
                    nc.vector.tensor_copy(ctx_nat_bf[:], ctx_nat[:])
                    if _DEBUG:
                        nc.sync.dma_start(dbg_ctx_d.ap(), ctx_nat[:])
                    nc.sync.dma_start(cc_ctx_in.ap(), ctx_nat_bf[:])
                    nc.gpsimd.collective_compute(
                        "AllGather", ALU.bypass, replica_groups=rgroups,
                        ins=[cc_ctx_in.ap().opt()],
                        outs=[cc_ctx_out.ap().opt()])
                    ctxn = work.tile([128, U], BF16)
                    nc.sync.dma_start(ctxn[:], cc_ctx_out.ap())
                    for c in range(PC):
                        ct_ps = psC.tile([128, 128], BF16, tag="ct")
                        nc.tensor.matmul(ct_ps[:],
                                         ctxn[:, c * 128:(c + 1) * 128],
                                         id128b[:], is_transpose=True)
                        nc.vector.tensor_copy(ctxT[:, c, :], ct_ps[:])

            # ---- LSTM + fc phase (bigA space reused) ----
            with tc.tile_pool(name="bigB", bufs=1) as bigB, \
                 tc.tile_pool(name="lwork", bufs=1) as lwork:
                fcw = bigB.tile([128, PC, FCP], BF16)
                for c in range(PC):
                    for hh in range(2):
                        sl = slice(hh * (FCP // 2), (hh + 1) * (FCP // 2))
                        nc.sync.dma_start(fcw[:, c, sl],
                                          fcw_d.ap()[:, c, sl])

                for li in range(4):
                    kc = 10 if li == 0 else 16
                    xtail = embT if li == 0 else hT
                    with tc.tile_pool(name=f"psL{li}", bufs=1,
                                      space="PSUM") as psL:
                        z_ps = psL.tile([128, 3 * GS], F32, tag="z")
                        for k in range(kc):
                            lhsT = (ctxT[:, k, :] if k < PC
                                    else xtail[:, k - PC, :])
                            nc.tensor.matmul(z_ps[:], lhsT, lws[li][:, k, :],
                                             start=(k == 0),
                                             stop=(k == kc - 1))
                        i_s = lwork.tile([128, GS], F32, tag="i_s")
                        g_t = lwork.tile([128, GS], F32, tag="g_t")
                        o_s = lwork.tile([128, GS], F32, tag="o_s")
                        c_f = lwork.tile([128, GS], F32, tag="c_f")
                        c_t = lwork.tile([128, GS], F32, tag="c_t")
                        h_n = lwork.tile([128, GS], F32, tag="h_n")
                        nc.scalar.activation(i_s[:], z_ps[:, 0:GS],
                                             AF.Sigmoid,
                                             bias=lbs[li][:, 0:1])
                        nc.scalar.activation(g_t[:], z_ps[:, GS:2 * GS],
                                             AF.Tanh, bias=lbs[li][:, 1:2])
                        nc.scalar.activation(o_s[:], z_ps[:, 2 * GS:3 * GS],
                                             AF.Sigmoid,
                                             bias=lbs[li][:, 2:3])
                        nc.vector.tensor_mul(c_f[:], i_s[:], g_t[:])
                        nc.scalar.activation(c_t[:], c_f[:], AF.Tanh)
                        nc.vector.tensor_mul(h_n[:], o_s[:], c_t[:])
                        # h^T shard for the partition-axis AllGather
                        ht_ps = psL.tile([128, 128], F32, tag="ht")
                        nc.tensor.matmul(ht_ps[:], h_n[:], id128[:],
                                         is_transpose=True)
                        hts_bf = lwork.tile([128, 128], BF16, tag="hts_bf")
                        nc.vector.tensor_copy(hts_bf[:], ht_ps[:])
                        if li == 3:
                            hts_f = lwork.tile([128, 128], F32, tag="hts_f")
                            nc.vector.tensor_copy(hts_f[:], ht_ps[:])
                            nc.sync.dma_start(hsh_d.ap(), hts_f[:])
                    nc.sync.dma_start(cc_h_in[li].ap(), hts_bf[:])
                    nc.gpsimd.collective_compute(
                        "AllGather", ALU.bypass, replica_groups=rgroups,
                        ins=[cc_h_in[li].ap().opt()],
                        outs=[cc_h_out[li].ap().opt()])
                    nc.sync.dma_start(
                        hT[:],
                        cc_h_out[li].ap().rearrange("(c p) b -> p c b", p=128))
                    if _DEBUG and li == 0:
                        dbgh = lwork.tile([128, PC * 128], F32, tag="dbgh")
                        nc.vector.tensor_copy(
                            dbgh[:].rearrange("p (c b) -> p c b", b=128),
                            hT[:])
                        nc.sync.dma_start(dbg_h1_d.ap(), dbgh[:])

                # ---- fc: logits = h @ fc_w + fc_b (natural output) ----
                lg_sb = lwork.tile([128, FCP], F32, tag="lg")
                with tc.tile_pool(name="psF", bufs=3, space="PSUM") as psF:
                    for nb in range(FCP // 512):
                        lg_ps = psF.tile([128, 512], F32, tag="lgp")
                        sl = slice(nb * 512, (nb + 1) * 512)
                        for k in range(PC):
                            nc.tensor.matmul(lg_ps[:], hT[:, k, :],
                                             fcw[:, k, sl],
                                             start=(k == 0), stop=False)
                        nc.tensor.matmul(lg_ps[:], ones128[:],
                                         fcb[0:1, sl], start=False, stop=True)
                        nc.scalar.copy(lg_sb[:, sl], lg_ps[:])
                        nc.sync.dma_start(logits_d.ap()[:, sl],
                                          lg_sb[:, sl])

    nc.compile()
    return nc


_NC_CACHE = None


def _get_nc():
    global _NC_CACHE
    if _NC_CACHE is None:
        _NC_CACHE = _build()
    return _NC_CACHE


def _prep_inputs(x, hidden, enc_output, W1_w, W1_b, W2_w, W2_b, V_w, V_b, emb,
                 l1_W, l1_b, l2_W, l2_b, l3_W, l3_b, l4_W, l4_b, fc_w, fc_b):
    """Host-side sharding/layout/casting. Returns in_maps for the 8 cores."""
    x = np.asarray(x).reshape(B).astype(np.int64)
    hidden = np.asarray(hidden, dtype=np.float32)
    enc_output = np.asarray(enc_output, dtype=np.float32)

    # shared (replicated) tensors
    w1 = _chunked(_to_bf16(W1_w))                  # [128, 8, U]
    w1b = _to_bf16(np.asarray(W1_b)).reshape(1, U)
    w2 = _chunked(_to_bf16(W2_w))
    w2bT = np.ascontiguousarray(
        np.asarray(W2_b, dtype=np.float32).reshape(PC, 128).T)
    vw = np.ascontiguousarray(
        _to_bf16(np.asarray(V_w).reshape(U)).reshape(PC, 128).T)
    # V_b shifts every score equally -> softmax-invariant; dropped.
    emb_bf = _to_bf16(emb)                          # [V, E]
    xw = np.ascontiguousarray(x.reshape(B, 1).astype(np.int32))
    fcb_full = np.asarray(fc_b, dtype=np.float32)

    lw_all, lb_all = [], []
    for W, bb in ((l1_W, l1_b), (l2_W, l2_b), (l3_W, l3_b), (l4_W, l4_b)):
        lw_all.append(_to_bf16(W))
        lb_all.append(np.asarray(bb, dtype=np.float32))

    in_maps = []
    for k in range(NC_N):
        bsl = slice(k * BS, (k + 1) * BS)
        enc_c = _to_bf16(enc_output[bsl]).reshape(BSS, U)
        encT = _chunked(np.ascontiguousarray(enc_c.T))      # [128, 8, 4096]
        hidT = _chunked(np.ascontiguousarray(
            _to_bf16(hidden[bsl]).T))                       # [128, 8, 16]

        m = {
            "encT": encT, "hidT": hidT,
            "w1": w1, "w1b": w1b, "w2": w2, "w2bT": w2bT, "vw": vw,
            "emb": emb_bf, "xidx": xw,
            "fcb": np.zeros((1, FCP), _BF),
        }
        m["fcb"][0, :FCC] = _to_bf16(fcb_full[k * FCC:(k + 1) * FCC])
        fcw_pad = np.zeros((U, FCP), _BF)
        fcw_pad[:, :FCC] = _to_bf16(
            np.asarray(fc_w)[:, k * FCC:(k + 1) * FCC])
        m["fcw"] = _chunked(fcw_pad)

        for i, (Wb, bb) in enumerate(zip(lw_all, lb_all), start=1):
            cols = np.concatenate(
                [Wb[:, g * U + k * GS:g * U + (k + 1) * GS]
                 for g in (0, 2, 3)], axis=1)               # i, g, o
            m[f"lw{i}"] = _chunked(np.ascontiguousarray(cols))
            m[f"lb{i}"] = np.ascontiguousarray(np.stack(
                [bb[g * U + k * GS:g * U + (k + 1) * GS]
                 for g in (0, 2, 3)], axis=1))              # [128, 3]
        in_maps.append(m)
    return in_maps


def _run(in_maps, trace=False):
    nc = _get_nc()
    return run_bass_kernel_spmd(
        nc, in_maps, core_ids=list(range(NC_N)), trace=trace)


def _assemble(results):
    logits = np.empty((B, V), np.float32)
    h = np.empty((B, U), np.float32)
    for k in range(NC_N):
        logits[:, k * FCC:(k + 1) * FCC] = results[k]["logits"][:, :FCC]
        h[:, k * GS:(k + 1) * GS] = results[k]["hsh"].T
    return logits, h


def kernel(**inputs):
    in_maps = _prep_inputs(**inputs)
    res = _run(in_maps, trace=False)
    return _assemble(res.results)


def kernel_traced(**inputs):
    """Like kernel() but with NTFF profiling; returns (outputs, exec_time_ns)."""
    in_maps = _prep_inputs(**inputs)
    res = _run(in_maps, trace=True)
    return _assemble(res.results), res.exec_time_ns
